# revision 20
# baseline (speedup 1.0000x reference)
"""Trainium2 Bass kernel for the masked-relu multi-head attention module.

Math (per batch b):
    qh = relu(q @ Wq.T + bq); kh, vh likewise
    scores = (qh/sqrt(D)) @ kh.T + mask        [per head]
    attn   = relu(softmax(scores) + mask2)
    out    = relu((attn @ vh)_concat @ Wo.T + bo)

Sharding: 8 cores = (batch b in 0..1) x (query block qb in 0..3).
Each core handles 512 queries of one batch, all 16 heads, all 2048 keys.
Each core projects kh/vh only for its OWN 512 tokens; the full khT/vh are
assembled with an AllGather over the 4-core batch group.

Device-side layout: scores are computed TRANSPOSED, [keys_part,
queries_free], which makes both attention matmuls transpose-free:
  scoresT = khT_chunk-as-lhsT @ qhT          (both [dim, token] layouts)
  outT    = vh-as-lhsT @ attn_T              (vh natural [token, dim])
The additive score mask becomes a multiplicative exp(mask) (computed once
per core, reused by all 16 heads); the softmax denominator (a
partition-axis sum in this layout) comes from a ones-vector matmul on the
PE, reshaped through a small DRAM bounce for the reciprocal. The
normalize + mask2 + relu + attn@v stage of head-pair N is emitted during
pair N+1 so the PE's in-order queue never stalls on the reciprocal chain.
All host-side work is pure layout (transpose / slice / cast / concat).

Compute dtype: bf16 operands with fp32 PSUM accumulation (validated
end-to-end ~5e-3 max rel err vs the fp32 reference).
"""

import sys

sys.path.insert(0, "/opt/trn_rl_repo")

import ml_dtypes
import numpy as np

from concourse import mybir
import concourse.bass as bass
import concourse.tile as tile
from concourse import bacc
from concourse.bass import ds, ts
from concourse.bass_utils import run_bass_kernel_spmd

B, S, E, H, D = 2, 2048, 1024, 16, 64
NCORES = 8
QB = NCORES // B            # query blocks per batch
NQ = S // QB                # queries per core (512)
P = 128
EC = E // P                 # 8 e-chunks
TC = S // P                 # 16 key chunks
SCALE = 1.0 / 8.0           # 1/sqrt(D)
GROUPS = [[0, 1, 2, 3], [4, 5, 6, 7]]

F32 = mybir.dt.float32
BF16 = mybir.dt.bfloat16
NPBF = ml_dtypes.bfloat16


def _emit(tc, io):
    """Emit the per-core program. io: dict of DRAM APs."""
    from contextlib import ExitStack

    nc = tc.nc
    Relu = mybir.ActivationFunctionType.Relu
    Exp = mybir.ActivationFunctionType.Exp

    with ExitStack() as ctx:
        # ---------------- constants ----------------
        cpool = ctx.enter_context(tc.tile_pool(name="const", bufs=1))
        ones128 = cpool.tile([P, 1], BF16)
        nc.vector.memset(ones128[:], 1.0)
        ones1b = cpool.tile([1, P], BF16)
        nc.vector.memset(ones1b[:], 1.0)
        ones1f = cpool.tile([1, P], F32)
        nc.vector.memset(ones1f[:], 1.0)

        id8 = cpool.tile([P, P], BF16)
        from concourse.masks import make_identity
        nc.gpsimd.memset(id8[:], 0.0)
        nc.gpsimd.affine_select(
            out=id8[:], in_=id8[:], compare_op=mybir.AluOpType.not_equal,
            fill=8.0, base=0, pattern=[[-1, P]], channel_multiplier=1)

        bq_t = cpool.tile([P, EC], F32)
        nc.sync.dma_start(bq_t[:], io["bq"].rearrange("(j p) -> p j", p=P))
        bk_t = cpool.tile([P, EC], F32)
        nc.sync.dma_start(bk_t[:], io["bk"].rearrange("(j p) -> p j", p=P))
        bo_t = cpool.tile([P, EC], F32)
        nc.sync.dma_start(bo_t[:], io["bo"].rearrange("(j p) -> p j", p=P))
        bv_t = cpool.tile([1, E], BF16)
        nc.sync.dma_start(bv_t[:], io["bv"].rearrange("(o e) -> o e", o=1))

        # long-lived activations (all bf16)
        rpool = ctx.enter_context(tc.tile_pool(name="resident", bufs=1))
        qhT = rpool.tile([P, EC, NQ], BF16)          # [dim, q]       8 KB/par
        headcat = rpool.tile([P, EC, NQ], BF16)      # [dim, q]       8 KB/par
        maskT = rpool.tile([P, TC, NQ], BF16)        # maskT         16 KB/par
        m2T = rpool.tile([P, TC, NQ], BF16)          # mask2T        16 KB/par

        dram = ctx.enter_context(tc.tile_pool(name="dram", bufs=1, space="DRAM"))
        dbounce = ctx.enter_context(tc.tile_pool(name="dbounce", bufs=2, space="DRAM"))

        # all input loads have no deps; they stream on the sync queue and are
        # ordered by first use (weights/x first - emitted in the proj block)
        def load_masks():
            for g in range(TC // 2):
                nc.sync.dma_start(
                    maskT[:, ts(g, 2), :],
                    io["maskT"].rearrange("(c p) q -> p c q", p=P)[:, ts(g, 2), :])
            for g in range(TC // 2):
                nc.sync.dma_start(
                    m2T[:, ts(g, 2), :],
                    io["mask2T"].rearrange("(c p) q -> p c q", p=P)[:, ts(g, 2), :])

        # ---------------- projections (own 512 tokens only) ----------------
        khT_part = dram.tile([E, NQ], BF16)          # this core's khT slice
        vh_part = dram.tile([NQ, E], BF16)           # this core's vh slice
        khT_ag = dram.tile([QB, E, NQ], BF16)
        vh_ag = dram.tile([QB, NQ, E], BF16)

        with tc.tile_pool(name="wt", bufs=2) as wpool, \
             tc.tile_pool(name="xt", bufs=2) as xpool, \
             tc.tile_pool(name="pout", bufs=2) as opool, \
             tc.tile_pool(name="pps", bufs=4, space="PSUM") as ppsum:

            def load_w(name):
                w_t = wpool.tile([P, EC, E], BF16, tag="w", name="w_t")
                for e in range(EC):
                    nc.sync.dma_start(
                        w_t[:, e, :],
                        io[name].rearrange("(eo p) d -> p eo d", p=P)[:, e, :])
                return w_t

            def load_x(dst, name):
                for e in range(EC):
                    nc.sync.dma_start(
                        dst[:, e, :],
                        io[name].rearrange("(eo p) t -> p eo t", p=P)[:, e, :])

            # k projection -> khT_part, then AllGather early
            wk_t = load_w("wkT")
            xk_t = xpool.tile([P, EC, NQ], BF16, tag="x", name="xk_t")
            load_x(xk_t, "kT")
            kp = opool.tile([P, EC, NQ], BF16, tag="kp", name="kp")
            for j in range(EC):
                ps = ppsum.tile([P, NQ], F32, tag="ps", name="ps")
                for e in range(EC):
                    nc.tensor.matmul(ps[:], wk_t[:, e, ts(j, P)], xk_t[:, e, :],
                                     start=(e == 0), stop=(e == EC - 1))
                nc.scalar.activation(kp[:, j, :], ps[:], Relu, bias=bk_t[:, ds(j, 1)])
            nc.gpsimd.dma_start(
                khT_part[:].rearrange("(jo p) t -> p jo t", p=P), kp[:])
            nc.gpsimd.collective_compute(
                "AllGather", mybir.AluOpType.bypass, replica_groups=GROUPS,
                ins=[khT_part.opt()], outs=[khT_ag.opt()])

            # v projection -> vh_part [tokens, dim]; bias rides a rank-1
            # ones-row matmul (it is along the free axis here).
            wv_t = load_w("wvT")
            xv_t = xpool.tile([P, EC, NQ], BF16, tag="x", name="xv_t")
            load_x(xv_t, "vT")
            vp = opool.tile([P, NQ // P, E], BF16, tag="vp", name="vp")
            for tc2 in range(NQ // P):              # 4 token chunks of 128
                for n in range(E // NQ):            # 2 output-dim halves of 512
                    ps = ppsum.tile([P, NQ], F32, tag="ps", name="ps")
                    for e in range(EC):
                        nc.tensor.matmul(ps[:], xv_t[:, e, ts(tc2, P)],
                                         wv_t[:, e, ts(n, NQ)],
                                         start=(e == 0), stop=False)
                    nc.tensor.matmul(ps[:], ones1b[:], bv_t[:, ts(n, NQ)],
                                     start=False, stop=True)
                    nc.scalar.activation(vp[:, tc2, ts(n, NQ)], ps[:], Relu)
            nc.gpsimd.dma_start(
                vh_part[:].rearrange("(c p) d -> p c d", p=P), vp[:])
            nc.gpsimd.collective_compute(
                "AllGather", mybir.AluOpType.bypass, replica_groups=GROUPS,
                ins=[vh_part.opt()], outs=[vh_ag.opt()])

            # q projection -> qhT resident
            wq_t = load_w("wqT")
            xq_t = xpool.tile([P, EC, NQ], BF16, tag="x", name="xq_t")
            load_x(xq_t, "qT")
            for j in range(EC):
                ps = ppsum.tile([P, NQ], F32, tag="ps", name="ps")
                for e in range(EC):
                    nc.tensor.matmul(ps[:], wq_t[:, e, ts(j, P)], xq_t[:, e, :],
                                     start=(e == 0), stop=(e == EC - 1))
                nc.scalar.activation(qhT[:, j, :], ps[:], Relu, bias=bq_t[:, ds(j, 1)])
            load_masks()

        # ---------------- attention ----------------
        with tc.tile_pool(name="p", bufs=2) as ppool, \
             tc.tile_pool(name="kv", bufs=2) as kvpool, \
             tc.tile_pool(name="work", bufs=2) as wk, \
             tc.tile_pool(name="invd", bufs=2) as ivpool, \
             tc.tile_pool(name="sps", bufs=2, space="PSUM") as spsum, \
             tc.tile_pool(name="dps", bufs=1, space="PSUM") as dpsum, \
             tc.tile_pool(name="ops", bufs=1, space="PSUM") as opsum:

            spart = [ds(0, D), ds(D, D)]

            def load_pair(pair):
                khp = kvpool.tile([P, QB, NQ], BF16, tag="kh", name="khp")
                nc.gpsimd.dma_start(
                    khp[:],
                    khT_ag[:, ds(pair * P, P), :].rearrange("g p t -> p g t"))
                vhp = kvpool.tile([P, TC, P], BF16, tag="vh", name="vhp")
                for g in range(QB):
                    nc.gpsimd.dma_start(
                        vhp[:, ds(g * (TC // QB), TC // QB), :],
                        vh_ag[g].rearrange("(c p) d -> p c d", p=P)[:, :, ds(pair * P, P)])
                return khp, vhp

            def emit_av(pair, p_pair, iv_pair, vhp):
                """normalize + mask2 + relu + attn@v for `pair` (delayed one
                pair so the PE queue never waits on the reciprocal chain)."""
                ib = []
                for hh in range(2):
                    b_ps = dpsum.tile([P, NQ], F32, tag=f"d{hh}", name=f"b_ps{hh}")
                    nc.tensor.matmul(b_ps[:], ones1f[:], iv_pair[hh][:],
                                     start=True, stop=True)
                    ib_t = ivpool.tile([P, NQ], BF16, tag=f"ib{hh}", name=f"ib{hh}")
                    nc.vector.tensor_copy(ib_t[:], b_ps[:])
                    ib.append(ib_t)
                o_ps = [opsum.tile([D, NQ], F32, tag=f"o{hh}", name=f"o_ps{hh}")
                        for hh in range(2)]
                for g in range(TC // 2):
                    for hh in range(2):
                        ibb = ib[hh][:, None, :].broadcast_to([P, 2, NQ])
                        p2 = wk.tile([P, 2, NQ], BF16, tag="p2", name="p2")
                        nc.vector.tensor_mul(p2[:], p_pair[hh][:, ts(g, 2), :], ibb)
                        w_t = wk.tile([P, 2, NQ], BF16, tag="w", name="w_t")
                        nc.vector.tensor_add(w_t[:], p2[:], m2T[:, ts(g, 2), :])
                        nc.vector.tensor_scalar_max(w_t[:], w_t[:], 0.0)
                        for cc in range(2):
                            c = 2 * g + cc
                            nc.tensor.matmul(o_ps[hh][:],
                                             vhp[:, c, ds(hh * D, D)],
                                             w_t[:, cc, :],
                                             start=(c == 0), stop=(c == TC - 1))
                for hh in range(2):
                    nc.scalar.copy(headcat[spart[hh], pair, :], o_ps[hh][:])

            prev = None
            nextkv = load_pair(0)
            for pair in range(H // 2):               # two heads per 128-row block
                khp, vhp = nextkv
                if pair + 1 < H // 2:
                    nextkv = load_pair(pair + 1)
                # scores (mask rides the matmul group) + exp
                p_t = [ppool.tile([P, TC, NQ], BF16, tag=f"p{hh}", name=f"p{hh}")
                       for hh in range(2)]
                for g in range(TC // 2):             # two key-chunks per psum tile
                    for hh in range(2):
                        s_ps = spsum.tile([P, 2, NQ], F32, tag="s", name="s_ps")
                        for cc in range(2):
                            c = 2 * g + cc
                            nc.tensor.matmul(
                                s_ps[:, cc, :], id8[:], maskT[:, c, :],
                                start=True, stop=False)
                            nc.tensor.matmul(
                                s_ps[:, cc, :],
                                khp[spart[hh], c // QB, ds((c % QB) * P, P)],
                                qhT[spart[hh], pair, :], start=False, stop=True)
                        psl = p_t[hh][:, ts(g, 2), :]
                        nc.scalar.activation(psl, s_ps[:], Exp, scale=SCALE)

                # softmax denominators via ones-matmul; 1/d via DRAM-bounce
                iv = []
                for hh in range(2):
                    d_ps = dpsum.tile([1, NQ], F32, tag=f"d{hh}", name=f"d_ps{hh}")
                    for c in range(TC):
                        nc.tensor.matmul(d_ps[:], ones128[:], p_t[hh][:, c, :],
                                         start=(c == 0), stop=(c == TC - 1))
                    d_sb = ivpool.tile([1, NQ], F32, tag="dsb", name="dsb")
                    nc.vector.tensor_copy(d_sb[:], d_ps[:])
                    d_dram = dbounce.tile([NQ], F32, tag="dd", name="dd")
                    nc.gpsimd.dma_start(d_dram[:].rearrange("(o q) -> o q", o=1), d_sb[:])
                    d_r = ivpool.tile([P, NQ // P], F32, tag="dr", name="dr")
                    nc.gpsimd.dma_start(d_r[:], d_dram[:].rearrange("(p f) -> p f", p=P))
                    iv_r = ivpool.tile([P, NQ // P], F32, tag="ivr", name="ivr")
                    nc.vector.reciprocal(iv_r[:], d_r[:])
                    iv_dram = dbounce.tile([NQ], F32, tag="ivd", name="ivd")
                    nc.gpsimd.dma_start(iv_dram[:].rearrange("(p f) -> p f", p=P), iv_r[:])
                    iv_f = ivpool.tile([1, NQ], F32, tag=f"ivf{hh}", name=f"ivf{hh}")
                    nc.gpsimd.dma_start(iv_f[:], iv_dram[:].rearrange("(o q) -> o q", o=1))
                    iv.append(iv_f)

                if prev is not None:
                    emit_av(prev[0], prev[1], prev[2], prev[3])
                prev = (pair, p_t, iv, vhp)
            emit_av(prev[0], prev[1], prev[2], prev[3])

        # ---------------- output projection ----------------
        with tc.tile_pool(name="wo", bufs=1) as wopool, \
             tc.tile_pool(name="ops2", bufs=4, space="PSUM") as opsum2, \
             tc.tile_pool(name="oout", bufs=4) as oopool:
            wo_t = wopool.tile([P, EC, E], BF16)
            nc.sync.dma_start(wo_t[:], io["woT"].rearrange("(eo p) d -> p eo d", p=P))
            for j in range(EC):
                ps = opsum2.tile([P, NQ], F32, tag="ps", name="ps")
                for e in range(EC):
                    nc.tensor.matmul(ps[:], wo_t[:, e, ts(j, P)], headcat[:, e, :],
                                     start=(e == 0), stop=(e == EC - 1))
                o_t = oopool.tile([P, NQ], F32, tag="o", name="o_t")
                nc.scalar.activation(o_t[:], ps[:], Relu, bias=bo_t[:, ds(j, 1)])
                nc.sync.dma_start(
                    io["outT"].rearrange("(jo p) q -> p jo q", p=P)[:, j, :], o_t[:])


_PROGRAM = None


def _build_program():
    global _PROGRAM
    if _PROGRAM is not None:
        return _PROGRAM
    nc = bacc.Bacc("TRN2", target_bir_lowering=False, debug=False,
                   num_devices=NCORES)
    io = {}
    def inp(name, shape, dt=BF16):
        io[name] = nc.dram_tensor(name, shape, dt, kind="ExternalInput").ap()
    inp("qT", [E, NQ])
    inp("kT", [E, NQ])
    inp("vT", [E, NQ])
    inp("maskT", [S, NQ])
    inp("mask2T", [S, NQ])
    for w in ("wqT", "wkT", "wvT", "woT"):
        inp(w, [E, E])
    for b in ("bq", "bk", "bo"):
        inp(b, [E], F32)
    inp("bv", [E], BF16)
    io["outT"] = nc.dram_tensor("outT", [E, NQ], F32, kind="ExternalOutput").ap()

    with tile.TileContext(nc) as tc:
        _emit(tc, io)
    nc.compile()
    _PROGRAM = (nc, io)
    return _PROGRAM


def kernel(q, k, v, mask, mask2, Wq, bq, Wk, bk, Wv, bv, Wo, bo, _trace=False):
    nc, _ = _build_program()

    def bf(x):
        return np.ascontiguousarray(x, dtype=NPBF)

    wqT = bf(Wq.T)
    wkT = bf(Wk.T)
    wvT = bf(Wv.T)
    woT = bf(Wo.T)

    in_maps = []
    for c in range(NCORES):
        b, qb = divmod(c, QB)
        rows = slice(qb * NQ, (qb + 1) * NQ)
        in_maps.append({
            "qT": bf(q[b, rows, :].T),
            "kT": bf(k[b, rows, :].T),
            "vT": bf(v[b, rows, :].T),
            "maskT": bf(mask[b, rows, :].T),
            "mask2T": bf(mask2[b, rows, :].T),
            "wqT": wqT, "wkT": wkT, "wvT": wvT, "woT": woT,
            "bq": np.ascontiguousarray(bq, dtype=np.float32),
            "bk": np.ascontiguousarray(bk, dtype=np.float32),
            "bo": np.ascontiguousarray(bo, dtype=np.float32),
            "bv": bf(bv),
        })

    res = run_bass_kernel_spmd(nc, in_maps, core_ids=list(range(NCORES)),
                               trace=_trace)

    out = np.empty((B, S, E), dtype=np.float32)
    for c in range(NCORES):
        b, qb = divmod(c, QB)
        out[b, qb * NQ:(qb + 1) * NQ, :] = res.results[c]["outT"].T
    if _trace:
        kernel.last_results = res
    return out


# revision 22
# speedup vs baseline: 1.1326x; 1.1326x over previous
"""Trainium2 Bass kernel for the masked-relu multi-head attention module.

Math (per batch b):
    qh = relu(q @ Wq.T + bq); kh, vh likewise
    scores = (qh/sqrt(D)) @ kh.T + mask        [per head]
    attn   = relu(softmax(scores) + mask2)
    out    = relu((attn @ vh)_concat @ Wo.T + bo)

Sharding: 8 cores = (batch b in 0..1) x (query block qb in 0..3).
Each core handles 512 queries of one batch, all 16 heads, all 2048 keys.
Each core projects kh/vh only for its OWN 512 tokens; the full khT/vh are
assembled with an AllGather over the 4-core batch group.

Device-side layout: scores are computed TRANSPOSED, [keys_part,
queries_free], which makes both attention matmuls transpose-free:
  scoresT = khT_chunk-as-lhsT @ qhT          (both [dim, token] layouts)
  outT    = vh-as-lhsT @ attn_T              (vh natural [token, dim])
The additive score mask becomes a multiplicative exp(mask) (computed once
per core, reused by all 16 heads); the softmax denominator (a
partition-axis sum in this layout) comes from a ones-vector matmul on the
PE, reshaped through a small DRAM bounce for the reciprocal. The
normalize + mask2 + relu + attn@v stage of head-pair N is emitted during
pair N+1 so the PE's in-order queue never stalls on the reciprocal chain.
All host-side work is pure layout (transpose / slice / cast / concat).

Compute dtype: bf16 operands with fp32 PSUM accumulation (validated
end-to-end ~5e-3 max rel err vs the fp32 reference).
"""

import sys

sys.path.insert(0, "/opt/trn_rl_repo")

import ml_dtypes
import numpy as np

from concourse import mybir
import concourse.bass as bass
import concourse.tile as tile
from concourse import bacc
from concourse.bass import ds, ts
from concourse.bass_utils import run_bass_kernel_spmd

B, S, E, H, D = 2, 2048, 1024, 16, 64
NCORES = 8
QB = NCORES // B            # query blocks per batch
NQ = S // QB                # queries per core (512)
P = 128
EC = E // P                 # 8 e-chunks
TC = S // P                 # 16 key chunks
SCALE = 1.0 / 8.0           # 1/sqrt(D)
GROUPS = [[0, 1, 2, 3], [4, 5, 6, 7]]

F32 = mybir.dt.float32
BF16 = mybir.dt.bfloat16
NPBF = ml_dtypes.bfloat16


def _emit(tc, io):
    """Emit the per-core program. io: dict of DRAM APs."""
    from contextlib import ExitStack

    nc = tc.nc
    Relu = mybir.ActivationFunctionType.Relu
    Exp = mybir.ActivationFunctionType.Exp

    with ExitStack() as ctx:
        # ---------------- constants ----------------
        cpool = ctx.enter_context(tc.tile_pool(name="const", bufs=1))
        ones128 = cpool.tile([P, 1], BF16)
        nc.vector.memset(ones128[:], 1.0)
        ones1b = cpool.tile([1, P], BF16)
        nc.vector.memset(ones1b[:], 1.0)
        ones1f = cpool.tile([1, P], F32)
        nc.vector.memset(ones1f[:], 1.0)

        id8 = cpool.tile([P, P], BF16)
        from concourse.masks import make_identity
        nc.gpsimd.memset(id8[:], 0.0)
        nc.gpsimd.affine_select(
            out=id8[:], in_=id8[:], compare_op=mybir.AluOpType.not_equal,
            fill=8.0, base=0, pattern=[[-1, P]], channel_multiplier=1)

        bq_t = cpool.tile([P, EC], F32)
        nc.sync.dma_start(bq_t[:], io["bq"].rearrange("(j p) -> p j", p=P))
        bk_t = cpool.tile([P, EC], F32)
        nc.sync.dma_start(bk_t[:], io["bk"].rearrange("(j p) -> p j", p=P))
        bo_t = cpool.tile([P, EC], F32)
        nc.sync.dma_start(bo_t[:], io["bo"].rearrange("(j p) -> p j", p=P))
        bv_t = cpool.tile([1, E], BF16)
        nc.sync.dma_start(bv_t[:], io["bv"].rearrange("(o e) -> o e", o=1))

        # long-lived activations (all bf16)
        rpool = ctx.enter_context(tc.tile_pool(name="resident", bufs=1))
        qhT = rpool.tile([P, EC, NQ], BF16)          # [dim, q]       8 KB/par
        headcat = rpool.tile([P, EC, NQ], BF16)      # [dim, q]       8 KB/par
        maskT = rpool.tile([P, TC, NQ], BF16)        # maskT         16 KB/par
        m2T = rpool.tile([P, TC, NQ], BF16)          # mask2T        16 KB/par

        dram = ctx.enter_context(tc.tile_pool(name="dram", bufs=1, space="DRAM"))
        dbounce = ctx.enter_context(tc.tile_pool(name="dbounce", bufs=2, space="DRAM"))

        # all input loads have no deps; they stream on the sync queue and are
        # ordered by first use (weights/x first - emitted in the proj block)
        def load_masks():
            for g in range(TC // 2):
                nc.sync.dma_start(
                    maskT[:, ts(g, 2), :],
                    io["maskT"].rearrange("(c p) q -> p c q", p=P)[:, ts(g, 2), :])
            for g in range(TC // 2):
                nc.sync.dma_start(
                    m2T[:, ts(g, 2), :],
                    io["mask2T"].rearrange("(c p) q -> p c q", p=P)[:, ts(g, 2), :])

        # ---------------- projections (own 512 tokens only) ----------------
        khT_part = dram.tile([E, NQ], BF16)          # this core's khT slice
        vh_part = dram.tile([NQ, E], BF16)           # this core's vh slice
        khT_ag = dram.tile([QB, E, NQ], BF16)
        vh_ag = dram.tile([QB, NQ, E], BF16)

        with tc.tile_pool(name="wt", bufs=2) as wpool, \
             tc.tile_pool(name="xt", bufs=2) as xpool, \
             tc.tile_pool(name="pout", bufs=2) as opool, \
             tc.tile_pool(name="pps", bufs=4, space="PSUM") as ppsum:

            def load_w(name):
                w_t = wpool.tile([P, EC, E], BF16, tag="w", name="w_t")
                for e in range(EC):
                    nc.sync.dma_start(
                        w_t[:, e, :],
                        io[name].rearrange("(eo p) d -> p eo d", p=P)[:, e, :])
                return w_t

            def load_x(dst, name):
                for e in range(EC):
                    nc.sync.dma_start(
                        dst[:, e, :],
                        io[name].rearrange("(eo p) t -> p eo t", p=P)[:, e, :])

            # k projection -> khT_part, then AllGather early
            wk_t = load_w("wkT")
            xk_t = xpool.tile([P, EC, NQ], BF16, tag="x", name="xk_t")
            load_x(xk_t, "kT")
            kp = opool.tile([P, EC, NQ], BF16, tag="kp", name="kp")
            for j in range(EC):
                ps = ppsum.tile([P, NQ], F32, tag="ps", name="ps")
                for e in range(EC):
                    nc.tensor.matmul(ps[:], wk_t[:, e, ts(j, P)], xk_t[:, e, :],
                                     start=(e == 0), stop=(e == EC - 1))
                nc.scalar.activation(kp[:, j, :], ps[:], Relu, bias=bk_t[:, ds(j, 1)])
            nc.gpsimd.dma_start(
                khT_part[:].rearrange("(jo p) t -> p jo t", p=P), kp[:])
            nc.gpsimd.collective_compute(
                "AllGather", mybir.AluOpType.bypass, replica_groups=GROUPS,
                ins=[khT_part.opt()], outs=[khT_ag.opt()])

            # v projection -> vh_part [tokens, dim]; bias rides a rank-1
            # ones-row matmul (it is along the free axis here).
            wv_t = load_w("wvT")
            xv_t = xpool.tile([P, EC, NQ], BF16, tag="x", name="xv_t")
            load_x(xv_t, "vT")
            vp = opool.tile([P, NQ // P, E], BF16, tag="vp", name="vp")
            for tc2 in range(NQ // P):              # 4 token chunks of 128
                for n in range(E // NQ):            # 2 output-dim halves of 512
                    ps = ppsum.tile([P, NQ], F32, tag="ps", name="ps")
                    for e in range(EC):
                        nc.tensor.matmul(ps[:], xv_t[:, e, ts(tc2, P)],
                                         wv_t[:, e, ts(n, NQ)],
                                         start=(e == 0), stop=False)
                    nc.tensor.matmul(ps[:], ones1b[:], bv_t[:, ts(n, NQ)],
                                     start=False, stop=True)
                    nc.scalar.activation(vp[:, tc2, ts(n, NQ)], ps[:], Relu)
            nc.gpsimd.dma_start(
                vh_part[:].rearrange("(c p) d -> p c d", p=P), vp[:])
            nc.gpsimd.collective_compute(
                "AllGather", mybir.AluOpType.bypass, replica_groups=GROUPS,
                ins=[vh_part.opt()], outs=[vh_ag.opt()])

            # q projection -> qhT resident
            wq_t = load_w("wqT")
            xq_t = xpool.tile([P, EC, NQ], BF16, tag="x", name="xq_t")
            load_x(xq_t, "qT")
            for j in range(EC):
                ps = ppsum.tile([P, NQ], F32, tag="ps", name="ps")
                for e in range(EC):
                    nc.tensor.matmul(ps[:], wq_t[:, e, ts(j, P)], xq_t[:, e, :],
                                     start=(e == 0), stop=(e == EC - 1))
                nc.scalar.activation(qhT[:, j, :], ps[:], Relu, bias=bq_t[:, ds(j, 1)])
            load_masks()

        # ---------------- attention ----------------
        with tc.tile_pool(name="p", bufs=2) as ppool, \
             tc.tile_pool(name="kv", bufs=2) as kvpool, \
             tc.tile_pool(name="work", bufs=2) as wk, \
             tc.tile_pool(name="invd", bufs=2) as ivpool, \
             tc.tile_pool(name="sps", bufs=2, space="PSUM") as spsum, \
             tc.tile_pool(name="dps", bufs=1, space="PSUM") as dpsum, \
             tc.tile_pool(name="ops", bufs=1, space="PSUM") as opsum:

            spart = [ds(0, D), ds(D, D)]

            def load_pair(pair):
                khp = kvpool.tile([P, QB, NQ], BF16, tag="kh", name="khp")
                nc.gpsimd.dma_start(
                    khp[:],
                    khT_ag[:, ds(pair * P, P), :].rearrange("g p t -> p g t"))
                vhp = kvpool.tile([P, TC, P], BF16, tag="vh", name="vhp", bufs=3)
                for g in range(QB):
                    nc.gpsimd.dma_start(
                        vhp[:, ds(g * (TC // QB), TC // QB), :],
                        vh_ag[g].rearrange("(c p) d -> p c d", p=P)[:, :, ds(pair * P, P)])
                return khp, vhp

            def emit_av_prologue(prev):
                """broadcast 1/d (tiny rank-1 matmuls; inputs long ready)."""
                ib = []
                for hh in range(2):
                    b_ps = dpsum.tile([P, NQ], F32, tag=f"d{hh}", name=f"b_ps{hh}")
                    nc.tensor.matmul(b_ps[:], ones1f[:], prev[2][hh][:],
                                     start=True, stop=True)
                    ib_t = ivpool.tile([P, NQ], BF16, tag=f"ib{hh}", name=f"ib{hh}")
                    nc.vector.tensor_copy(ib_t[:], b_ps[:])
                    ib.append(ib_t)
                o_ps = [opsum.tile([D, NQ], F32, tag=f"o{hh}", name=f"o_ps{hh}")
                        for hh in range(2)]
                return ib, o_ps

            def emit_av_block(prev, ib, o_ps, g):
                """normalize + mask2 + relu + attn@v for chunk-pair g of the
                PREVIOUS head pair (interleaved into the current pair)."""
                _, p_prev, _, vhp_prev = prev
                for hh in range(2):
                    ibb = ib[hh][:, None, :].broadcast_to([P, 2, NQ])
                    p2 = wk.tile([P, 2, NQ], BF16, tag="p2", name="p2")
                    nc.vector.tensor_mul(p2[:], p_prev[hh][:, ts(g, 2), :], ibb)
                    w_t = wk.tile([P, 2, NQ], BF16, tag="w", name="w_t")
                    nc.vector.tensor_add(w_t[:], p2[:], m2T[:, ts(g, 2), :])
                    nc.vector.tensor_scalar_max(w_t[:], w_t[:], 0.0)
                    for cc in range(2):
                        c = 2 * g + cc
                        nc.tensor.matmul(o_ps[hh][:],
                                         vhp_prev[:, c, ds(hh * D, D)],
                                         w_t[:, cc, :],
                                         start=(c == 0), stop=(c == TC - 1))

            def emit_av_epilogue(prev, o_ps):
                for hh in range(2):
                    nc.scalar.copy(headcat[spart[hh], prev[0], :], o_ps[hh][:])

            def emit_d_chunks(d_ps, p_t, g):
                """ones-matmul accumulation of chunks 2g, 2g+1 for both heads."""
                for hh in range(2):
                    for cc in range(2):
                        c = 2 * g + cc
                        nc.tensor.matmul(d_ps[hh][:], ones128[:], p_t[hh][:, c, :],
                                         start=(c == 0), stop=(c == TC - 1))

            def emit_bounce(d_ps, hh):
                """PSUM d -> reciprocal -> [1, NQ] 1/d via DRAM reshape."""
                d_sb = ivpool.tile([1, NQ], F32, tag="dsb", name="dsb")
                nc.vector.tensor_copy(d_sb[:], d_ps[hh][:])
                d_dram = dbounce.tile([NQ], F32, tag="dd", name="dd")
                nc.gpsimd.dma_start(d_dram[:].rearrange("(o q) -> o q", o=1), d_sb[:])
                d_r = ivpool.tile([P, NQ // P], F32, tag="dr", name="dr")
                nc.gpsimd.dma_start(d_r[:], d_dram[:].rearrange("(p f) -> p f", p=P))
                iv_r = ivpool.tile([P, NQ // P], F32, tag="ivr", name="ivr")
                nc.vector.reciprocal(iv_r[:], d_r[:])
                iv_dram = dbounce.tile([NQ], F32, tag="ivd", name="ivd")
                nc.gpsimd.dma_start(iv_dram[:].rearrange("(p f) -> p f", p=P), iv_r[:])
                iv_f = ivpool.tile([1, NQ], F32, tag=f"ivf{hh}", name=f"ivf{hh}")
                nc.gpsimd.dma_start(iv_f[:], iv_dram[:].rearrange("(o q) -> o q", o=1))
                return iv_f

            prev = None
            nextkv = load_pair(0)
            for pair in range(H // 2):               # two heads per 128-row block
                khp, vhp = nextkv
                if pair + 1 < H // 2:
                    nextkv = load_pair(pair + 1)
                p_t = [ppool.tile([P, TC, NQ], BF16, tag=f"p{hh}", name=f"p{hh}")
                       for hh in range(2)]
                d_ps = [dpsum.tile([1, NQ], F32, tag=f"d{hh}", name=f"d_ps{hh}")
                        for hh in range(2)]
                if prev is not None:
                    ib, o_ps = emit_av_prologue(prev)
                # chunk-interleaved emission: the PE queue alternates between
                # scores (gated by exp recycling s_ps), the d ones-matmuls
                # (gated by exp one chunk back), and the previous pair's
                # attn@v (inputs all ready) - so it never stalls.
                for g in range(TC // 2):
                    for hh in range(2):
                        s_ps = spsum.tile([P, 2, NQ], F32, tag="s", name="s_ps")
                        for cc in range(2):
                            c = 2 * g + cc
                            nc.tensor.matmul(
                                s_ps[:, cc, :], id8[:], maskT[:, c, :],
                                start=True, stop=False)
                            nc.tensor.matmul(
                                s_ps[:, cc, :],
                                khp[spart[hh], c // QB, ds((c % QB) * P, P)],
                                qhT[spart[hh], pair, :], start=False, stop=True)
                        psl = p_t[hh][:, ts(g, 2), :]
                        nc.scalar.activation(psl, s_ps[:], Exp, scale=SCALE)
                    if g >= 1:
                        emit_d_chunks(d_ps, p_t, g - 1)
                    if prev is not None:
                        emit_av_block(prev, ib, o_ps, g)
                emit_d_chunks(d_ps, p_t, TC // 2 - 1)
                iv = [emit_bounce(d_ps, hh) for hh in range(2)]
                if prev is not None:
                    emit_av_epilogue(prev, o_ps)
                prev = (pair, p_t, iv, vhp)
            ib, o_ps = emit_av_prologue(prev)
            for g in range(TC // 2):
                emit_av_block(prev, ib, o_ps, g)
            emit_av_epilogue(prev, o_ps)

        # ---------------- output projection ----------------
        with tc.tile_pool(name="wo", bufs=1) as wopool, \
             tc.tile_pool(name="ops2", bufs=4, space="PSUM") as opsum2, \
             tc.tile_pool(name="oout", bufs=4) as oopool:
            wo_t = wopool.tile([P, EC, E], BF16)
            nc.sync.dma_start(wo_t[:], io["woT"].rearrange("(eo p) d -> p eo d", p=P))
            for j in range(EC):
                ps = opsum2.tile([P, NQ], F32, tag="ps", name="ps")
                for e in range(EC):
                    nc.tensor.matmul(ps[:], wo_t[:, e, ts(j, P)], headcat[:, e, :],
                                     start=(e == 0), stop=(e == EC - 1))
                o_t = oopool.tile([P, NQ], F32, tag="o", name="o_t")
                nc.scalar.activation(o_t[:], ps[:], Relu, bias=bo_t[:, ds(j, 1)])
                nc.sync.dma_start(
                    io["outT"].rearrange("(jo p) q -> p jo q", p=P)[:, j, :], o_t[:])


_PROGRAM = None


def _build_program():
    global _PROGRAM
    if _PROGRAM is not None:
        return _PROGRAM
    nc = bacc.Bacc("TRN2", target_bir_lowering=False, debug=False,
                   num_devices=NCORES)
    io = {}
    def inp(name, shape, dt=BF16):
        io[name] = nc.dram_tensor(name, shape, dt, kind="ExternalInput").ap()
    inp("qT", [E, NQ])
    inp("kT", [E, NQ])
    inp("vT", [E, NQ])
    inp("maskT", [S, NQ])
    inp("mask2T", [S, NQ])
    for w in ("wqT", "wkT", "wvT", "woT"):
        inp(w, [E, E])
    for b in ("bq", "bk", "bo"):
        inp(b, [E], F32)
    inp("bv", [E], BF16)
    io["outT"] = nc.dram_tensor("outT", [E, NQ], F32, kind="ExternalOutput").ap()

    with tile.TileContext(nc) as tc:
        _emit(tc, io)
    nc.compile()
    _PROGRAM = (nc, io)
    return _PROGRAM


def kernel(q, k, v, mask, mask2, Wq, bq, Wk, bk, Wv, bv, Wo, bo, _trace=False):
    nc, _ = _build_program()

    def bf(x):
        return np.ascontiguousarray(x, dtype=NPBF)

    wqT = bf(Wq.T)
    wkT = bf(Wk.T)
    wvT = bf(Wv.T)
    woT = bf(Wo.T)

    in_maps = []
    for c in range(NCORES):
        b, qb = divmod(c, QB)
        rows = slice(qb * NQ, (qb + 1) * NQ)
        in_maps.append({
            "qT": bf(q[b, rows, :].T),
            "kT": bf(k[b, rows, :].T),
            "vT": bf(v[b, rows, :].T),
            "maskT": bf(mask[b, rows, :].T),
            "mask2T": bf(mask2[b, rows, :].T),
            "wqT": wqT, "wkT": wkT, "wvT": wvT, "woT": woT,
            "bq": np.ascontiguousarray(bq, dtype=np.float32),
            "bk": np.ascontiguousarray(bk, dtype=np.float32),
            "bo": np.ascontiguousarray(bo, dtype=np.float32),
            "bv": bf(bv),
        })

    res = run_bass_kernel_spmd(nc, in_maps, core_ids=list(range(NCORES)),
                               trace=_trace)

    out = np.empty((B, S, E), dtype=np.float32)
    for c in range(NCORES):
        b, qb = divmod(c, QB)
        out[b, qb * NQ:(qb + 1) * NQ, :] = res.results[c]["outT"].T
    if _trace:
        kernel.last_results = res
    return out


# revision 23
# speedup vs baseline: 1.3063x; 1.1533x over previous
"""Trainium2 Bass kernel for the masked-relu multi-head attention module.

Math (per batch b):
    qh = relu(q @ Wq.T + bq); kh, vh likewise
    scores = (qh/sqrt(D)) @ kh.T + mask        [per head]
    attn   = relu(softmax(scores) + mask2)
    out    = relu((attn @ vh)_concat @ Wo.T + bo)

Sharding: 8 cores = (batch b in 0..1) x (query block qb in 0..3).
Each core handles 512 queries of one batch, all 16 heads, all 2048 keys.
Each core projects kh/vh only for its OWN 512 tokens; the full khT/vh are
assembled with an AllGather over the 4-core batch group.

Device-side layout: scores are computed TRANSPOSED, [keys_part,
queries_free], which makes both attention matmuls transpose-free:
  scoresT = khT_chunk-as-lhsT @ qhT          (both [dim, token] layouts)
  outT    = vh-as-lhsT @ attn_T              (vh natural [token, dim])
The additive score mask becomes a multiplicative exp(mask) (computed once
per core, reused by all 16 heads); the softmax denominator (a
partition-axis sum in this layout) comes from a ones-vector matmul on the
PE, reshaped through a small DRAM bounce for the reciprocal. The
normalize + mask2 + relu + attn@v stage of head-pair N is emitted during
pair N+1 so the PE's in-order queue never stalls on the reciprocal chain.
All host-side work is pure layout (transpose / slice / cast / concat).

Compute dtype: bf16 operands with fp32 PSUM accumulation (validated
end-to-end ~5e-3 max rel err vs the fp32 reference).
"""

import sys

sys.path.insert(0, "/opt/trn_rl_repo")

import ml_dtypes
import numpy as np

from concourse import mybir
import concourse.bass as bass
import concourse.tile as tile
from concourse import bacc
from concourse.bass import ds, ts
from concourse.bass_utils import run_bass_kernel_spmd

B, S, E, H, D = 2, 2048, 1024, 16, 64
NCORES = 8
QB = NCORES // B            # query blocks per batch
NQ = S // QB                # queries per core (512)
P = 128
EC = E // P                 # 8 e-chunks
TC = S // P                 # 16 key chunks
SCALE = 1.0 / 8.0           # 1/sqrt(D)
GROUPS = [[0, 1, 2, 3], [4, 5, 6, 7]]

F32 = mybir.dt.float32
BF16 = mybir.dt.bfloat16
NPBF = ml_dtypes.bfloat16


def _emit(tc, io):
    """Emit the per-core program. io: dict of DRAM APs."""
    from contextlib import ExitStack

    nc = tc.nc
    Relu = mybir.ActivationFunctionType.Relu
    Exp = mybir.ActivationFunctionType.Exp

    with ExitStack() as ctx:
        # ---------------- constants ----------------
        cpool = ctx.enter_context(tc.tile_pool(name="const", bufs=1))
        ones128 = cpool.tile([P, 1], BF16)
        nc.vector.memset(ones128[:], 1.0)
        ones1b = cpool.tile([1, P], BF16)
        nc.vector.memset(ones1b[:], 1.0)
        ones1f = cpool.tile([1, P], F32)
        nc.vector.memset(ones1f[:], 1.0)

        bq_t = cpool.tile([P, EC], F32)
        nc.sync.dma_start(bq_t[:], io["bq"].rearrange("(j p) -> p j", p=P))
        bk_t = cpool.tile([P, EC], F32)
        nc.sync.dma_start(bk_t[:], io["bk"].rearrange("(j p) -> p j", p=P))
        bo_t = cpool.tile([P, EC], F32)
        nc.sync.dma_start(bo_t[:], io["bo"].rearrange("(j p) -> p j", p=P))
        bv_t = cpool.tile([1, E], BF16)
        nc.sync.dma_start(bv_t[:], io["bv"].rearrange("(o e) -> o e", o=1))

        # long-lived activations (all bf16)
        rpool = ctx.enter_context(tc.tile_pool(name="resident", bufs=1))
        qhT = rpool.tile([P, EC, NQ], BF16)          # [dim, q]       8 KB/par
        headcat = rpool.tile([P, EC, NQ], BF16)      # [dim, q]       8 KB/par
        eM = rpool.tile([P, TC, NQ], BF16)           # exp(maskT)    16 KB/par
        m2T = rpool.tile([P, TC, NQ], BF16)          # mask2T        16 KB/par

        dram = ctx.enter_context(tc.tile_pool(name="dram", bufs=1, space="DRAM"))
        dbounce = ctx.enter_context(tc.tile_pool(name="dbounce", bufs=2, space="DRAM"))

        # all input loads have no deps; they stream on the sync queue and are
        # ordered by first use (weights/x first - emitted in the proj block)
        def load_masks(mlp):
            for g in range(TC // 2):
                mt = mlp.tile([P, 2, NQ], BF16, tag="mt", name="mt")
                nc.sync.dma_start(
                    mt[:], io["maskT"].rearrange("(c p) q -> p c q", p=P)[:, ts(g, 2), :])
                nc.scalar.activation(eM[:, ts(g, 2), :], mt[:], Exp)
            for g in range(TC // 2):
                nc.sync.dma_start(
                    m2T[:, ts(g, 2), :],
                    io["mask2T"].rearrange("(c p) q -> p c q", p=P)[:, ts(g, 2), :])

        # ---------------- projections (own 512 tokens only) ----------------
        khT_part = dram.tile([E, NQ], BF16)          # this core's khT slice
        vh_part = dram.tile([NQ, E], BF16)           # this core's vh slice
        khT_ag = dram.tile([QB, E, NQ], BF16)
        vh_ag = dram.tile([QB, NQ, E], BF16)

        with tc.tile_pool(name="wt", bufs=2) as wpool, \
             tc.tile_pool(name="xt", bufs=2) as xpool, \
             tc.tile_pool(name="pout", bufs=2) as opool, \
             tc.tile_pool(name="pps", bufs=4, space="PSUM") as ppsum:

            def load_w(name):
                w_t = wpool.tile([P, EC, E], BF16, tag="w", name="w_t")
                for e in range(EC):
                    nc.sync.dma_start(
                        w_t[:, e, :],
                        io[name].rearrange("(eo p) d -> p eo d", p=P)[:, e, :])
                return w_t

            def load_x(dst, name):
                for e in range(EC):
                    nc.sync.dma_start(
                        dst[:, e, :],
                        io[name].rearrange("(eo p) t -> p eo t", p=P)[:, e, :])

            # k projection -> khT_part, then AllGather early
            wk_t = load_w("wkT")
            xk_t = xpool.tile([P, EC, NQ], BF16, tag="x", name="xk_t")
            load_x(xk_t, "kT")
            kp = opool.tile([P, EC, NQ], BF16, tag="kp", name="kp")
            for j in range(EC):
                ps = ppsum.tile([P, NQ], F32, tag="ps", name="ps")
                for e in range(EC):
                    nc.tensor.matmul(ps[:], wk_t[:, e, ts(j, P)], xk_t[:, e, :],
                                     start=(e == 0), stop=(e == EC - 1))
                nc.scalar.activation(kp[:, j, :], ps[:], Relu, bias=bk_t[:, ds(j, 1)])
                nc.gpsimd.dma_start(
                    khT_part[:].rearrange("(jo p) t -> p jo t", p=P)[:, j, :],
                    kp[:, j, :])
            nc.gpsimd.collective_compute(
                "AllGather", mybir.AluOpType.bypass, replica_groups=GROUPS,
                ins=[khT_part.opt()], outs=[khT_ag.opt()])

            # v projection -> vh_part [tokens, dim]; bias rides a rank-1
            # ones-row matmul (it is along the free axis here).
            wv_t = load_w("wvT")
            xv_t = xpool.tile([P, EC, NQ], BF16, tag="x", name="xv_t")
            load_x(xv_t, "vT")
            vp = opool.tile([P, NQ // P, E], BF16, tag="vp", name="vp")
            for tc2 in range(NQ // P):              # 4 token chunks of 128
                for n in range(E // NQ):            # 2 output-dim halves of 512
                    ps = ppsum.tile([P, NQ], F32, tag="ps", name="ps")
                    for e in range(EC):
                        nc.tensor.matmul(ps[:], xv_t[:, e, ts(tc2, P)],
                                         wv_t[:, e, ts(n, NQ)],
                                         start=(e == 0), stop=False)
                    nc.tensor.matmul(ps[:], ones1b[:], bv_t[:, ts(n, NQ)],
                                     start=False, stop=True)
                    nc.scalar.activation(vp[:, tc2, ts(n, NQ)], ps[:], Relu)
                    nc.gpsimd.dma_start(
                        vh_part[:].rearrange("(c p) d -> p c d", p=P)[:, tc2, ts(n, NQ)],
                        vp[:, tc2, ts(n, NQ)])
            nc.gpsimd.collective_compute(
                "AllGather", mybir.AluOpType.bypass, replica_groups=GROUPS,
                ins=[vh_part.opt()], outs=[vh_ag.opt()])

            # q projection -> qhT resident
            wq_t = load_w("wqT")
            xq_t = xpool.tile([P, EC, NQ], BF16, tag="x", name="xq_t")
            load_x(xq_t, "qT")
            for j in range(EC):
                ps = ppsum.tile([P, NQ], F32, tag="ps", name="ps")
                for e in range(EC):
                    nc.tensor.matmul(ps[:], wq_t[:, e, ts(j, P)], xq_t[:, e, :],
                                     start=(e == 0), stop=(e == EC - 1))
                nc.scalar.activation(qhT[:, j, :], ps[:], Relu, bias=bq_t[:, ds(j, 1)])
            load_masks(xpool)

        # ---------------- attention ----------------
        with tc.tile_pool(name="p", bufs=2) as ppool, \
             tc.tile_pool(name="kv", bufs=2) as kvpool, \
             tc.tile_pool(name="work", bufs=2) as wk, \
             tc.tile_pool(name="invd", bufs=2) as ivpool, \
             tc.tile_pool(name="sps", bufs=2, space="PSUM") as spsum, \
             tc.tile_pool(name="dps", bufs=1, space="PSUM") as dpsum, \
             tc.tile_pool(name="ops", bufs=1, space="PSUM") as opsum:

            spart = [ds(0, D), ds(D, D)]

            def load_pair(pair):
                khp = kvpool.tile([P, QB, NQ], BF16, tag="kh", name="khp")
                nc.sync.dma_start(
                    khp[:],
                    khT_ag[:, ds(pair * P, P), :].rearrange("g p t -> p g t"))
                vhp = kvpool.tile([P, TC, P], BF16, tag="vh", name="vhp", bufs=3)
                for g in range(QB):
                    nc.sync.dma_start(
                        vhp[:, ds(g * (TC // QB), TC // QB), :],
                        vh_ag[g].rearrange("(c p) d -> p c d", p=P)[:, :, ds(pair * P, P)])
                return khp, vhp

            def emit_av_prologue(prev):
                """broadcast 1/d (tiny rank-1 matmuls; inputs long ready)."""
                ib = []
                for hh in range(2):
                    b_ps = dpsum.tile([P, NQ], F32, tag=f"d{hh}", name=f"b_ps{hh}")
                    nc.tensor.matmul(b_ps[:], ones1f[:], prev[2][hh][:],
                                     start=True, stop=True)
                    ib_t = ivpool.tile([P, NQ], BF16, tag=f"ib{hh}", name=f"ib{hh}")
                    nc.vector.tensor_copy(ib_t[:], b_ps[:])
                    ib.append(ib_t)
                o_ps = [opsum.tile([D, NQ], F32, tag=f"o{hh}", name=f"o_ps{hh}")
                        for hh in range(2)]
                return ib, o_ps

            def emit_av_block(prev, ib, o_ps, g):
                """normalize + mask2 + relu + attn@v for chunk-pair g of the
                PREVIOUS head pair (interleaved into the current pair)."""
                _, p_prev, _, vhp_prev = prev
                for hh in range(2):
                    ibb = ib[hh][:, None, :].broadcast_to([P, 2, NQ])
                    p2 = wk.tile([P, 2, NQ], BF16, tag="p2", name="p2")
                    nc.vector.tensor_mul(p2[:], p_prev[hh][:, ts(g, 2), :], ibb)
                    w_t = wk.tile([P, 2, NQ], BF16, tag="w", name="w_t")
                    nc.vector.tensor_add(w_t[:], p2[:], m2T[:, ts(g, 2), :])
                    nc.vector.tensor_scalar_max(w_t[:], w_t[:], 0.0)
                    for cc in range(2):
                        c = 2 * g + cc
                        nc.tensor.matmul(o_ps[hh][:],
                                         vhp_prev[:, c, ds(hh * D, D)],
                                         w_t[:, cc, :],
                                         start=(c == 0), stop=(c == TC - 1))

            def emit_av_epilogue(prev, o_ps):
                for hh in range(2):
                    nc.scalar.copy(headcat[spart[hh], prev[0], :], o_ps[hh][:])

            def emit_d_chunks(d_ps, p_t, g):
                """ones-matmul accumulation of chunks 2g, 2g+1 for both heads."""
                for hh in range(2):
                    for cc in range(2):
                        c = 2 * g + cc
                        nc.tensor.matmul(d_ps[hh][:], ones128[:], p_t[hh][:, c, :],
                                         start=(c == 0), stop=(c == TC - 1))

            def emit_bounce(d_ps, hh):
                """PSUM d -> reciprocal -> [1, NQ] 1/d via DRAM reshape."""
                d_sb = ivpool.tile([1, NQ], F32, tag="dsb", name="dsb")
                nc.vector.tensor_copy(d_sb[:], d_ps[hh][:])
                d_dram = dbounce.tile([NQ], F32, tag="dd", name="dd")
                nc.gpsimd.dma_start(d_dram[:].rearrange("(o q) -> o q", o=1), d_sb[:])
                d_r = ivpool.tile([P, NQ // P], F32, tag="dr", name="dr")
                nc.gpsimd.dma_start(d_r[:], d_dram[:].rearrange("(p f) -> p f", p=P))
                iv_r = ivpool.tile([P, NQ // P], F32, tag="ivr", name="ivr")
                nc.vector.reciprocal(iv_r[:], d_r[:])
                iv_dram = dbounce.tile([NQ], F32, tag="ivd", name="ivd")
                nc.gpsimd.dma_start(iv_dram[:].rearrange("(p f) -> p f", p=P), iv_r[:])
                iv_f = ivpool.tile([1, NQ], F32, tag=f"ivf{hh}", name=f"ivf{hh}")
                nc.gpsimd.dma_start(iv_f[:], iv_dram[:].rearrange("(o q) -> o q", o=1))
                return iv_f

            prev = None
            nextkv = load_pair(0)
            for pair in range(H // 2):               # two heads per 128-row block
                khp, vhp = nextkv
                if pair + 1 < H // 2:
                    nextkv = load_pair(pair + 1)
                p_t = [ppool.tile([P, TC, NQ], BF16, tag=f"p{hh}", name=f"p{hh}")
                       for hh in range(2)]
                d_ps = [dpsum.tile([1, NQ], F32, tag=f"d{hh}", name=f"d_ps{hh}")
                        for hh in range(2)]
                if prev is not None:
                    ib, o_ps = emit_av_prologue(prev)
                # chunk-interleaved emission: the PE queue alternates between
                # scores (gated by exp recycling s_ps), the d ones-matmuls
                # (gated by exp one chunk back), and the previous pair's
                # attn@v (inputs all ready) - so it never stalls.
                for g in range(TC // 2):
                    for hh in range(2):
                        s_ps = spsum.tile([P, 2, NQ], F32, tag="s", name="s_ps")
                        for cc in range(2):
                            c = 2 * g + cc
                            nc.tensor.matmul(
                                s_ps[:, cc, :],
                                khp[spart[hh], c // QB, ds((c % QB) * P, P)],
                                qhT[spart[hh], pair, :], start=True, stop=True)
                        psl = p_t[hh][:, ts(g, 2), :]
                        nc.scalar.activation(psl, s_ps[:], Exp, scale=SCALE)
                        nc.vector.tensor_mul(psl, psl, eM[:, ts(g, 2), :])
                    if g >= 1:
                        emit_d_chunks(d_ps, p_t, g - 1)
                    if prev is not None:
                        emit_av_block(prev, ib, o_ps, g)
                emit_d_chunks(d_ps, p_t, TC // 2 - 1)
                iv = [emit_bounce(d_ps, hh) for hh in range(2)]
                if prev is not None:
                    emit_av_epilogue(prev, o_ps)
                prev = (pair, p_t, iv, vhp)
            ib, o_ps = emit_av_prologue(prev)
            for g in range(TC // 2):
                emit_av_block(prev, ib, o_ps, g)
            emit_av_epilogue(prev, o_ps)

        # ---------------- output projection ----------------
        with tc.tile_pool(name="wo", bufs=1) as wopool, \
             tc.tile_pool(name="ops2", bufs=4, space="PSUM") as opsum2, \
             tc.tile_pool(name="oout", bufs=4) as oopool:
            wo_t = wopool.tile([P, EC, E], BF16)
            nc.sync.dma_start(wo_t[:], io["woT"].rearrange("(eo p) d -> p eo d", p=P))
            for j in range(EC):
                ps = opsum2.tile([P, NQ], F32, tag="ps", name="ps")
                for e in range(EC):
                    nc.tensor.matmul(ps[:], wo_t[:, e, ts(j, P)], headcat[:, e, :],
                                     start=(e == 0), stop=(e == EC - 1))
                o_t = oopool.tile([P, NQ], F32, tag="o", name="o_t")
                nc.scalar.activation(o_t[:], ps[:], Relu, bias=bo_t[:, ds(j, 1)])
                nc.sync.dma_start(
                    io["outT"].rearrange("(jo p) q -> p jo q", p=P)[:, j, :], o_t[:])


_PROGRAM = None


def _build_program():
    global _PROGRAM
    if _PROGRAM is not None:
        return _PROGRAM
    nc = bacc.Bacc("TRN2", target_bir_lowering=False, debug=False,
                   num_devices=NCORES)
    io = {}
    def inp(name, shape, dt=BF16):
        io[name] = nc.dram_tensor(name, shape, dt, kind="ExternalInput").ap()
    inp("qT", [E, NQ])
    inp("kT", [E, NQ])
    inp("vT", [E, NQ])
    inp("maskT", [S, NQ])
    inp("mask2T", [S, NQ])
    for w in ("wqT", "wkT", "wvT", "woT"):
        inp(w, [E, E])
    for b in ("bq", "bk", "bo"):
        inp(b, [E], F32)
    inp("bv", [E], BF16)
    io["outT"] = nc.dram_tensor("outT", [E, NQ], F32, kind="ExternalOutput").ap()

    with tile.TileContext(nc) as tc:
        _emit(tc, io)
    nc.compile()
    _PROGRAM = (nc, io)
    return _PROGRAM


def kernel(q, k, v, mask, mask2, Wq, bq, Wk, bk, Wv, bv, Wo, bo, _trace=False):
    nc, _ = _build_program()

    def bf(x):
        return np.ascontiguousarray(x, dtype=NPBF)

    wqT = bf(Wq.T)
    wkT = bf(Wk.T)
    wvT = bf(Wv.T)
    woT = bf(Wo.T)

    in_maps = []
    for c in range(NCORES):
        b, qb = divmod(c, QB)
        rows = slice(qb * NQ, (qb + 1) * NQ)
        in_maps.append({
            "qT": bf(q[b, rows, :].T),
            "kT": bf(k[b, rows, :].T),
            "vT": bf(v[b, rows, :].T),
            "maskT": bf(mask[b, rows, :].T),
            "mask2T": bf(mask2[b, rows, :].T),
            "wqT": wqT, "wkT": wkT, "wvT": wvT, "woT": woT,
            "bq": np.ascontiguousarray(bq, dtype=np.float32),
            "bk": np.ascontiguousarray(bk, dtype=np.float32),
            "bo": np.ascontiguousarray(bo, dtype=np.float32),
            "bv": bf(bv),
        })

    res = run_bass_kernel_spmd(nc, in_maps, core_ids=list(range(NCORES)),
                               trace=_trace)

    out = np.empty((B, S, E), dtype=np.float32)
    for c in range(NCORES):
        b, qb = divmod(c, QB)
        out[b, qb * NQ:(qb + 1) * NQ, :] = res.results[c]["outT"].T
    if _trace:
        kernel.last_results = res
    return out


# revision 24
# speedup vs baseline: 1.3703x; 1.0490x over previous
"""Trainium2 Bass kernel for the masked-relu multi-head attention module.

Math (per batch b):
    qh = relu(q @ Wq.T + bq); kh, vh likewise
    scores = (qh/sqrt(D)) @ kh.T + mask        [per head]
    attn   = relu(softmax(scores) + mask2)
    out    = relu((attn @ vh)_concat @ Wo.T + bo)

Sharding: 8 cores = (batch b in 0..1) x (query block qb in 0..3).
Each core handles 512 queries of one batch, all 16 heads, all 2048 keys.
Each core projects kh/vh only for its OWN 512 tokens; the full khT/vh are
assembled with an AllGather over the 4-core batch group.

Device-side layout: scores are computed TRANSPOSED, [keys_part,
queries_free], which makes both attention matmuls transpose-free:
  scoresT = khT_chunk-as-lhsT @ qhT          (both [dim, token] layouts)
  outT    = vh-as-lhsT @ attn_T              (vh natural [token, dim])
The additive score mask becomes a multiplicative exp(mask) (computed once
per core, reused by all 16 heads); the softmax denominator (a
partition-axis sum in this layout) comes from a ones-vector matmul on the
PE, reshaped through a small DRAM bounce for the reciprocal. The
normalize + mask2 + relu + attn@v stage of head-pair N is emitted during
pair N+1 so the PE's in-order queue never stalls on the reciprocal chain.
All host-side work is pure layout (transpose / slice / cast / concat).

Compute dtype: bf16 operands with fp32 PSUM accumulation (validated
end-to-end ~5e-3 max rel err vs the fp32 reference).
"""

import sys

sys.path.insert(0, "/opt/trn_rl_repo")

import ml_dtypes
import numpy as np

from concourse import mybir
import concourse.bass as bass
import concourse.tile as tile
from concourse import bacc
from concourse.bass import ds, ts
from concourse.bass_utils import run_bass_kernel_spmd

B, S, E, H, D = 2, 2048, 1024, 16, 64
NCORES = 8
QB = NCORES // B            # query blocks per batch
NQ = S // QB                # queries per core (512)
P = 128
EC = E // P                 # 8 e-chunks
TC = S // P                 # 16 key chunks
SCALE = 1.0 / 8.0           # 1/sqrt(D)
GROUPS = [[0, 1, 2, 3], [4, 5, 6, 7]]

F32 = mybir.dt.float32
BF16 = mybir.dt.bfloat16
NPBF = ml_dtypes.bfloat16


def _emit(tc, io):
    """Emit the per-core program. io: dict of DRAM APs."""
    from contextlib import ExitStack

    nc = tc.nc
    Relu = mybir.ActivationFunctionType.Relu
    Exp = mybir.ActivationFunctionType.Exp

    with ExitStack() as ctx:
        # ---------------- constants ----------------
        cpool = ctx.enter_context(tc.tile_pool(name="const", bufs=1))
        ones128 = cpool.tile([P, 1], BF16)
        nc.vector.memset(ones128[:], 1.0)
        ones1b = cpool.tile([1, P], BF16)
        nc.vector.memset(ones1b[:], 1.0)
        ones1f = cpool.tile([1, P], F32)
        nc.vector.memset(ones1f[:], 1.0)

        bq_t = cpool.tile([P, EC], F32)
        nc.sync.dma_start(bq_t[:], io["bq"].rearrange("(j p) -> p j", p=P))
        bk_t = cpool.tile([P, EC], F32)
        nc.sync.dma_start(bk_t[:], io["bk"].rearrange("(j p) -> p j", p=P))
        bo_t = cpool.tile([P, EC], F32)
        nc.sync.dma_start(bo_t[:], io["bo"].rearrange("(j p) -> p j", p=P))
        bv_t = cpool.tile([1, E], BF16)
        nc.sync.dma_start(bv_t[:], io["bv"].rearrange("(o e) -> o e", o=1))

        # long-lived activations (all bf16)
        rpool = ctx.enter_context(tc.tile_pool(name="resident", bufs=1))
        qhT = rpool.tile([P, EC, NQ], BF16)          # [dim, q]       8 KB/par
        headcat = rpool.tile([P, EC, NQ], BF16)      # [dim, q]       8 KB/par
        eM = rpool.tile([P, TC, NQ], BF16)           # exp(maskT)    16 KB/par
        m2T = rpool.tile([P, TC, NQ], BF16)          # mask2T        16 KB/par

        dram = ctx.enter_context(tc.tile_pool(name="dram", bufs=1, space="DRAM"))
        dbounce = ctx.enter_context(tc.tile_pool(name="dbounce", bufs=2, space="DRAM"))

        # all input loads have no deps; they stream on the sync queue and are
        # ordered by first use (weights/x first - emitted in the proj block)
        def load_masks(mlp):
            for g in range(TC // 2):
                mt = mlp.tile([P, 2, NQ], BF16, tag="mt", name="mt")
                nc.scalar.dma_start(
                    mt[:], io["maskT"].rearrange("(c p) q -> p c q", p=P)[:, ts(g, 2), :])
                nc.scalar.activation(eM[:, ts(g, 2), :], mt[:], Exp)
            for g in range(TC // 2):
                nc.scalar.dma_start(
                    m2T[:, ts(g, 2), :],
                    io["mask2T"].rearrange("(c p) q -> p c q", p=P)[:, ts(g, 2), :])

        # ---------------- projections (own 512 tokens only) ----------------
        khT_part = dram.tile([E, NQ], BF16)          # this core's khT slice
        vh_part = dram.tile([NQ, E], BF16)           # this core's vh slice
        khT_ag = dram.tile([QB, E, NQ], BF16)
        vh_ag = dram.tile([QB, NQ, E], BF16)

        with tc.tile_pool(name="wt", bufs=2) as wpool, \
             tc.tile_pool(name="xt", bufs=2) as xpool, \
             tc.tile_pool(name="pout", bufs=2) as opool, \
             tc.tile_pool(name="pps", bufs=4, space="PSUM") as ppsum:

            def load_w(name):
                w_t = wpool.tile([P, EC, E], BF16, tag="w", name="w_t")
                for e in range(EC):
                    nc.sync.dma_start(
                        w_t[:, e, :],
                        io[name].rearrange("(eo p) d -> p eo d", p=P)[:, e, :])
                return w_t

            def load_x(dst, name):
                for e in range(EC):
                    nc.sync.dma_start(
                        dst[:, e, :],
                        io[name].rearrange("(eo p) t -> p eo t", p=P)[:, e, :])

            # k projection -> khT_part, then AllGather early
            wk_t = load_w("wkT")
            xk_t = xpool.tile([P, EC, NQ], BF16, tag="x", name="xk_t")
            load_x(xk_t, "kT")
            kp = opool.tile([P, EC, NQ], BF16, tag="kp", name="kp")
            for j in range(EC):
                ps = ppsum.tile([P, NQ], F32, tag="ps", name="ps")
                for e in range(EC):
                    nc.tensor.matmul(ps[:], wk_t[:, e, ts(j, P)], xk_t[:, e, :],
                                     start=(e == 0), stop=(e == EC - 1))
                nc.scalar.activation(kp[:, j, :], ps[:], Relu, bias=bk_t[:, ds(j, 1)])
                nc.gpsimd.dma_start(
                    khT_part[:].rearrange("(jo p) t -> p jo t", p=P)[:, j, :],
                    kp[:, j, :])
            nc.gpsimd.collective_compute(
                "AllGather", mybir.AluOpType.bypass, replica_groups=GROUPS,
                ins=[khT_part.opt()], outs=[khT_ag.opt()])

            # v projection -> vh_part [tokens, dim]; bias rides a rank-1
            # ones-row matmul (it is along the free axis here).
            wv_t = load_w("wvT")
            xv_t = xpool.tile([P, EC, NQ], BF16, tag="x", name="xv_t")
            load_x(xv_t, "vT")
            vp = opool.tile([P, NQ // P, E], BF16, tag="vp", name="vp")
            for tc2 in range(NQ // P):              # 4 token chunks of 128
                for n in range(E // NQ):            # 2 output-dim halves of 512
                    ps = ppsum.tile([P, NQ], F32, tag="ps", name="ps")
                    for e in range(EC):
                        nc.tensor.matmul(ps[:], xv_t[:, e, ts(tc2, P)],
                                         wv_t[:, e, ts(n, NQ)],
                                         start=(e == 0), stop=False)
                    nc.tensor.matmul(ps[:], ones1b[:], bv_t[:, ts(n, NQ)],
                                     start=False, stop=True)
                    nc.scalar.activation(vp[:, tc2, ts(n, NQ)], ps[:], Relu)
                    nc.gpsimd.dma_start(
                        vh_part[:].rearrange("(c p) d -> p c d", p=P)[:, tc2, ts(n, NQ)],
                        vp[:, tc2, ts(n, NQ)])
            nc.gpsimd.collective_compute(
                "AllGather", mybir.AluOpType.bypass, replica_groups=GROUPS,
                ins=[vh_part.opt()], outs=[vh_ag.opt()])

            # q projection -> qhT resident
            wq_t = load_w("wqT")
            xq_t = xpool.tile([P, EC, NQ], BF16, tag="x", name="xq_t")
            load_x(xq_t, "qT")
            for j in range(EC):
                ps = ppsum.tile([P, NQ], F32, tag="ps", name="ps")
                for e in range(EC):
                    nc.tensor.matmul(ps[:], wq_t[:, e, ts(j, P)], xq_t[:, e, :],
                                     start=(e == 0), stop=(e == EC - 1))
                nc.scalar.activation(qhT[:, j, :], ps[:], Relu, bias=bq_t[:, ds(j, 1)])
            load_masks(xpool)

        # ---------------- attention ----------------
        with tc.tile_pool(name="p", bufs=2) as ppool, \
             tc.tile_pool(name="kv", bufs=2) as kvpool, \
             tc.tile_pool(name="work", bufs=3) as wk, \
             tc.tile_pool(name="invd", bufs=2) as ivpool, \
             tc.tile_pool(name="sps", bufs=2, space="PSUM") as spsum, \
             tc.tile_pool(name="dps", bufs=1, space="PSUM") as dpsum, \
             tc.tile_pool(name="ops", bufs=1, space="PSUM") as opsum:

            spart = [ds(0, D), ds(D, D)]

            def load_pair(pair):
                khp = kvpool.tile([P, QB, NQ], BF16, tag="kh", name="khp")
                nc.sync.dma_start(
                    khp[:],
                    khT_ag[:, ds(pair * P, P), :].rearrange("g p t -> p g t"))
                vhp = kvpool.tile([P, TC, P], BF16, tag="vh", name="vhp", bufs=3)
                for g in range(QB):
                    nc.sync.dma_start(
                        vhp[:, ds(g * (TC // QB), TC // QB), :],
                        vh_ag[g].rearrange("(c p) d -> p c d", p=P)[:, :, ds(pair * P, P)])
                return khp, vhp

            def emit_av_prologue(prev):
                """broadcast 1/d (tiny rank-1 matmuls; inputs long ready)."""
                ib = []
                for hh in range(2):
                    b_ps = dpsum.tile([P, NQ], F32, tag=f"d{hh}", name=f"b_ps{hh}")
                    nc.tensor.matmul(b_ps[:], ones1f[:], prev[2][hh][:],
                                     start=True, stop=True)
                    ib_t = ivpool.tile([P, NQ], BF16, tag=f"ib{hh}", name=f"ib{hh}")
                    nc.vector.tensor_copy(ib_t[:], b_ps[:])
                    ib.append(ib_t)
                # both heads share one PSUM bank, split on the partition axis
                o_ps = opsum.tile([P, NQ], F32, tag="o", name="o_ps")
                return ib, o_ps

            def emit_av_block(prev, ib, o_ps, g):
                """normalize + mask2 + relu + attn@v for chunk-pair g of the
                PREVIOUS head pair (interleaved into the current pair)."""
                _, p_prev, _, vhp_prev = prev
                for hh in range(2):
                    ibb = ib[hh][:, None, :].broadcast_to([P, 2, NQ])
                    half = p_prev[2 * hh + g // 4]
                    p2 = wk.tile([P, 2, NQ], BF16, tag="p2", name="p2")
                    nc.vector.tensor_mul(p2[:], half[:, ts(g % 4, 2), :], ibb)
                    w_t = wk.tile([P, 2, NQ], BF16, tag="w", name="w_t")
                    nc.vector.tensor_add(w_t[:], p2[:], m2T[:, ts(g, 2), :])
                    nc.vector.tensor_scalar_max(w_t[:], w_t[:], 0.0)
                    for cc in range(2):
                        c = 2 * g + cc
                        nc.tensor.matmul(o_ps[ds(hh * D, D), :],
                                         vhp_prev[:, c, ds(hh * D, D)],
                                         w_t[:, cc, :],
                                         start=(c == 0), stop=(c == TC - 1),
                                         skip_group_check=True)

            def emit_av_epilogue(prev, o_ps):
                nc.vector.tensor_copy(headcat[:, prev[0], :], o_ps[:])

            def emit_d_chunks(d_ps, p_t, g):
                """ones-matmul accumulation of chunks 2g, 2g+1 for both heads."""
                for hh in range(2):
                    for cc in range(2):
                        c = 2 * g + cc
                        half = p_t[2 * hh + g // 4]
                        nc.tensor.matmul(d_ps[hh][:], ones128[:],
                                         half[:, (g % 4) * 2 + cc, :],
                                         start=(c == 0), stop=(c == TC - 1))

            def emit_bounce(d_ps, hh):
                """PSUM d -> reciprocal -> [1, NQ] 1/d via DRAM reshape."""
                d_sb = ivpool.tile([1, NQ], F32, tag="dsb", name="dsb")
                nc.vector.tensor_copy(d_sb[:], d_ps[hh][:])
                d_dram = dbounce.tile([NQ], F32, tag="dd", name="dd")
                nc.gpsimd.dma_start(d_dram[:].rearrange("(o q) -> o q", o=1), d_sb[:])
                d_r = ivpool.tile([P, NQ // P], F32, tag="dr", name="dr")
                nc.gpsimd.dma_start(d_r[:], d_dram[:].rearrange("(p f) -> p f", p=P))
                iv_r = ivpool.tile([P, NQ // P], F32, tag="ivr", name="ivr")
                nc.vector.reciprocal(iv_r[:], d_r[:])
                iv_dram = dbounce.tile([NQ], F32, tag="ivd", name="ivd")
                nc.gpsimd.dma_start(iv_dram[:].rearrange("(p f) -> p f", p=P), iv_r[:])
                iv_f = ivpool.tile([1, NQ], F32, tag=f"ivf{hh}", name=f"ivf{hh}")
                nc.gpsimd.dma_start(iv_f[:], iv_dram[:].rearrange("(o q) -> o q", o=1))
                return iv_f

            prev = None
            nextkv = load_pair(0)
            for pair in range(H // 2):               # two heads per 128-row block
                khp, vhp = nextkv
                if pair + 1 < H // 2:
                    nextkv = load_pair(pair + 1)
                # p split into half-pair tiles so pair N+1's scores don't wait
                # on the full consumption of pair N-1's p
                p_t = [ppool.tile([P, TC // 2, NQ], BF16, tag=f"p{hh}{ab}",
                                  name=f"p{hh}{ab}")
                       for hh in range(2) for ab in range(2)]
                d_ps = [dpsum.tile([1, NQ], F32, tag=f"d{hh}", name=f"d_ps{hh}")
                        for hh in range(2)]
                # chunk-interleaved emission: the PE queue alternates between
                # scores (gated by exp recycling s_ps), the d ones-matmuls
                # (gated by exp one chunk back), and the previous pair's
                # attn@v (inputs all ready) - so it never stalls.
                for g in range(TC // 2):
                    for hh in range(2):
                        s_ps = spsum.tile([P, 2, NQ], F32, tag="s", name="s_ps")
                        for cc in range(2):
                            c = 2 * g + cc
                            nc.tensor.matmul(
                                s_ps[:, cc, :],
                                khp[spart[hh], c // QB, ds((c % QB) * P, P)],
                                qhT[spart[hh], pair, :], start=True, stop=True)
                        half = p_t[2 * hh + g // 4]
                        psl = half[:, ts(g % 4, 2), :]
                        nc.scalar.activation(psl, s_ps[:], Exp, scale=SCALE)
                        nc.vector.tensor_mul(psl, psl, eM[:, ts(g, 2), :])
                    if g == 1 and prev is not None:
                        ib, o_ps = emit_av_prologue(prev)
                    if g >= 1:
                        emit_d_chunks(d_ps, p_t, g - 1)
                    if g >= 1 and prev is not None:
                        emit_av_block(prev, ib, o_ps, g - 1)
                emit_d_chunks(d_ps, p_t, TC // 2 - 1)
                iv = [emit_bounce(d_ps, hh) for hh in range(2)]
                if prev is not None:
                    emit_av_block(prev, ib, o_ps, TC // 2 - 1)
                    emit_av_epilogue(prev, o_ps)
                prev = (pair, p_t, iv, vhp)
            ib, o_ps = emit_av_prologue(prev)
            for g in range(TC // 2):
                emit_av_block(prev, ib, o_ps, g)
            emit_av_epilogue(prev, o_ps)

        # ---------------- output projection ----------------
        with tc.tile_pool(name="wo", bufs=1) as wopool, \
             tc.tile_pool(name="ops2", bufs=4, space="PSUM") as opsum2, \
             tc.tile_pool(name="oout", bufs=4) as oopool:
            wo_t = wopool.tile([P, EC, E], BF16)
            nc.sync.dma_start(wo_t[:], io["woT"].rearrange("(eo p) d -> p eo d", p=P))
            for j in range(EC):
                ps = opsum2.tile([P, NQ], F32, tag="ps", name="ps")
                for e in range(EC):
                    nc.tensor.matmul(ps[:], wo_t[:, e, ts(j, P)], headcat[:, e, :],
                                     start=(e == 0), stop=(e == EC - 1))
                o_t = oopool.tile([P, NQ], F32, tag="o", name="o_t")
                nc.scalar.activation(o_t[:], ps[:], Relu, bias=bo_t[:, ds(j, 1)])
                nc.sync.dma_start(
                    io["outT"].rearrange("(jo p) q -> p jo q", p=P)[:, j, :], o_t[:])


_PROGRAM = None


def _build_program():
    global _PROGRAM
    if _PROGRAM is not None:
        return _PROGRAM
    nc = bacc.Bacc("TRN2", target_bir_lowering=False, debug=False,
                   num_devices=NCORES)
    io = {}
    def inp(name, shape, dt=BF16):
        io[name] = nc.dram_tensor(name, shape, dt, kind="ExternalInput").ap()
    inp("qT", [E, NQ])
    inp("kT", [E, NQ])
    inp("vT", [E, NQ])
    inp("maskT", [S, NQ])
    inp("mask2T", [S, NQ])
    for w in ("wqT", "wkT", "wvT", "woT"):
        inp(w, [E, E])
    for b in ("bq", "bk", "bo"):
        inp(b, [E], F32)
    inp("bv", [E], BF16)
    io["outT"] = nc.dram_tensor("outT", [E, NQ], F32, kind="ExternalOutput").ap()

    with tile.TileContext(nc) as tc:
        _emit(tc, io)
    nc.compile()
    _PROGRAM = (nc, io)
    return _PROGRAM


def kernel(q, k, v, mask, mask2, Wq, bq, Wk, bk, Wv, bv, Wo, bo, _trace=False):
    nc, _ = _build_program()

    def bf(x):
        return np.ascontiguousarray(x, dtype=NPBF)

    wqT = bf(Wq.T)
    wkT = bf(Wk.T)
    wvT = bf(Wv.T)
    woT = bf(Wo.T)

    in_maps = []
    for c in range(NCORES):
        b, qb = divmod(c, QB)
        rows = slice(qb * NQ, (qb + 1) * NQ)
        in_maps.append({
            "qT": bf(q[b, rows, :].T),
            "kT": bf(k[b, rows, :].T),
            "vT": bf(v[b, rows, :].T),
            "maskT": bf(mask[b, rows, :].T),
            "mask2T": bf(mask2[b, rows, :].T),
            "wqT": wqT, "wkT": wkT, "wvT": wvT, "woT": woT,
            "bq": np.ascontiguousarray(bq, dtype=np.float32),
            "bk": np.ascontiguousarray(bk, dtype=np.float32),
            "bo": np.ascontiguousarray(bo, dtype=np.float32),
            "bv": bf(bv),
        })

    res = run_bass_kernel_spmd(nc, in_maps, core_ids=list(range(NCORES)),
                               trace=_trace)

    out = np.empty((B, S, E), dtype=np.float32)
    for c in range(NCORES):
        b, qb = divmod(c, QB)
        out[b, qb * NQ:(qb + 1) * NQ, :] = res.results[c]["outT"].T
    if _trace:
        kernel.last_results = res
    return out


# revision 26
# speedup vs baseline: 1.3854x; 1.0110x over previous
"""Trainium2 Bass kernel for the masked-relu multi-head attention module.

Math (per batch b):
    qh = relu(q @ Wq.T + bq); kh, vh likewise
    scores = (qh/sqrt(D)) @ kh.T + mask        [per head]
    attn   = relu(softmax(scores) + mask2)
    out    = relu((attn @ vh)_concat @ Wo.T + bo)

Sharding: 8 cores = (batch b in 0..1) x (query block qb in 0..3).
Each core handles 512 queries of one batch, all 16 heads, all 2048 keys.
Each core projects kh/vh only for its OWN 512 tokens; the full khT/vh are
assembled with an AllGather over the 4-core batch group.

Device-side layout: scores are computed TRANSPOSED, [keys_part,
queries_free], which makes both attention matmuls transpose-free:
  scoresT = khT_chunk-as-lhsT @ qhT          (both [dim, token] layouts)
  outT    = vh-as-lhsT @ attn_T              (vh natural [token, dim])
The additive score mask becomes a multiplicative exp(mask) (computed once
per core, reused by all 16 heads); the softmax denominator (a
partition-axis sum in this layout) comes from a ones-vector matmul on the
PE, reshaped through a small DRAM bounce for the reciprocal. The
normalize + mask2 + relu + attn@v stage of head-pair N is emitted during
pair N+1 so the PE's in-order queue never stalls on the reciprocal chain.
All host-side work is pure layout (transpose / slice / cast / concat).

Compute dtype: bf16 operands with fp32 PSUM accumulation (validated
end-to-end ~5e-3 max rel err vs the fp32 reference).
"""

import sys

sys.path.insert(0, "/opt/trn_rl_repo")

import ml_dtypes
import numpy as np

from concourse import mybir
import concourse.bass as bass
import concourse.tile as tile
from concourse import bacc
from concourse.bass import ds, ts
from concourse.bass_utils import run_bass_kernel_spmd

B, S, E, H, D = 2, 2048, 1024, 16, 64
NCORES = 8
QB = NCORES // B            # query blocks per batch
NQ = S // QB                # queries per core (512)
P = 128
EC = E // P                 # 8 e-chunks
TC = S // P                 # 16 key chunks
SCALE = 1.0 / 8.0           # 1/sqrt(D)
GROUPS = [[0, 1, 2, 3], [4, 5, 6, 7]]

F32 = mybir.dt.float32
BF16 = mybir.dt.bfloat16
NPBF = ml_dtypes.bfloat16


def _emit(tc, io):
    """Emit the per-core program. io: dict of DRAM APs."""
    from contextlib import ExitStack

    nc = tc.nc
    Relu = mybir.ActivationFunctionType.Relu
    Exp = mybir.ActivationFunctionType.Exp

    with ExitStack() as ctx:
        # ---------------- constants ----------------
        cpool = ctx.enter_context(tc.tile_pool(name="const", bufs=1))
        ones128 = cpool.tile([P, 1], BF16)
        nc.vector.memset(ones128[:], 1.0)
        ones1b = cpool.tile([1, P], BF16)
        nc.vector.memset(ones1b[:], 1.0)
        ones1f = cpool.tile([1, P], F32)
        nc.vector.memset(ones1f[:], 1.0)

        bq_t = cpool.tile([P, EC], F32)
        nc.sync.dma_start(bq_t[:], io["bq"].rearrange("(j p) -> p j", p=P))
        bk_t = cpool.tile([P, EC], F32)
        nc.sync.dma_start(bk_t[:], io["bk"].rearrange("(j p) -> p j", p=P))
        bo_t = cpool.tile([P, EC], F32)
        nc.sync.dma_start(bo_t[:], io["bo"].rearrange("(j p) -> p j", p=P))
        bv_t = cpool.tile([1, E], BF16)
        nc.sync.dma_start(bv_t[:], io["bv"].rearrange("(o e) -> o e", o=1))

        # long-lived activations (all bf16)
        rpool = ctx.enter_context(tc.tile_pool(name="resident", bufs=1))
        qhT = rpool.tile([P, EC, NQ], BF16)          # [dim, q]       8 KB/par
        headcat = rpool.tile([P, EC, NQ], BF16)      # [dim, q]       8 KB/par
        eM = rpool.tile([P, TC, NQ], BF16)           # exp(maskT)    16 KB/par
        m2T = rpool.tile([P, TC, NQ], BF16)          # mask2T        16 KB/par

        dram = ctx.enter_context(tc.tile_pool(name="dram", bufs=1, space="DRAM"))
        dbounce = ctx.enter_context(tc.tile_pool(name="dbounce", bufs=2, space="DRAM"))

        # all input loads have no deps; they stream on the sync queue and are
        # ordered by first use (weights/x first - emitted in the proj block)
        def load_masks(mlp):
            for g in range(TC // 2):
                mt = mlp.tile([P, 2, NQ], BF16, tag="mt", name="mt")
                nc.scalar.dma_start(
                    mt[:], io["maskT"].rearrange("(c p) q -> p c q", p=P)[:, ts(g, 2), :])
                nc.scalar.activation(eM[:, ts(g, 2), :], mt[:], Exp)
            for g in range(TC // 2):
                nc.scalar.dma_start(
                    m2T[:, ts(g, 2), :],
                    io["mask2T"].rearrange("(c p) q -> p c q", p=P)[:, ts(g, 2), :])

        # ---------------- projections (own 512 tokens only) ----------------
        khT_part = dram.tile([E, NQ], BF16)          # this core's khT slice
        vh_part = dram.tile([NQ, E], BF16)           # this core's vh slice
        khT_ag = dram.tile([QB, E, NQ], BF16)
        vh_ag = dram.tile([QB, NQ, E], BF16)

        with tc.tile_pool(name="wt", bufs=2) as wpool, \
             tc.tile_pool(name="xt", bufs=2) as xpool, \
             tc.tile_pool(name="pout", bufs=2) as opool, \
             tc.tile_pool(name="pps", bufs=4, space="PSUM") as ppsum:

            def load_w(name):
                w_t = wpool.tile([P, EC, E], BF16, tag="w", name="w_t")
                for e in range(EC):
                    nc.sync.dma_start(
                        w_t[:, e, :],
                        io[name].rearrange("(eo p) d -> p eo d", p=P)[:, e, :])
                return w_t

            def load_x(dst, name):
                for e in range(EC):
                    nc.sync.dma_start(
                        dst[:, e, :],
                        io[name].rearrange("(eo p) t -> p eo t", p=P)[:, e, :])

            # k projection -> khT_part, then AllGather early
            wk_t = load_w("wkT")
            xk_t = xpool.tile([P, EC, NQ], BF16, tag="x", name="xk_t")
            load_x(xk_t, "kT")
            kp = opool.tile([P, EC, NQ], BF16, tag="kp", name="kp")
            for j in range(EC):
                ps = ppsum.tile([P, NQ], F32, tag="ps", name="ps")
                for e in range(EC):
                    nc.tensor.matmul(ps[:], wk_t[:, e, ts(j, P)], xk_t[:, e, :],
                                     start=(e == 0), stop=(e == EC - 1))
                nc.scalar.activation(kp[:, j, :], ps[:], Relu, bias=bk_t[:, ds(j, 1)])
                nc.gpsimd.dma_start(
                    khT_part[:].rearrange("(jo p) t -> p jo t", p=P)[:, j, :],
                    kp[:, j, :])
            nc.gpsimd.collective_compute(
                "AllGather", mybir.AluOpType.bypass, replica_groups=GROUPS,
                ins=[khT_part.opt()], outs=[khT_ag.opt()])

            # v projection -> vh_part [tokens, dim]; bias rides a rank-1
            # ones-row matmul (it is along the free axis here).
            wv_t = load_w("wvT")
            xv_t = xpool.tile([P, EC, NQ], BF16, tag="x", name="xv_t")
            load_x(xv_t, "vT")
            vp = opool.tile([P, NQ // P, E], BF16, tag="vp", name="vp")
            for tc2 in range(NQ // P):              # 4 token chunks of 128
                for n in range(E // NQ):            # 2 output-dim halves of 512
                    ps = ppsum.tile([P, NQ], F32, tag="ps", name="ps")
                    for e in range(EC):
                        nc.tensor.matmul(ps[:], xv_t[:, e, ts(tc2, P)],
                                         wv_t[:, e, ts(n, NQ)],
                                         start=(e == 0), stop=False)
                    nc.tensor.matmul(ps[:], ones1b[:], bv_t[:, ts(n, NQ)],
                                     start=False, stop=True)
                    nc.scalar.activation(vp[:, tc2, ts(n, NQ)], ps[:], Relu)
                    nc.gpsimd.dma_start(
                        vh_part[:].rearrange("(c p) d -> p c d", p=P)[:, tc2, ts(n, NQ)],
                        vp[:, tc2, ts(n, NQ)])
            nc.gpsimd.collective_compute(
                "AllGather", mybir.AluOpType.bypass, replica_groups=GROUPS,
                ins=[vh_part.opt()], outs=[vh_ag.opt()])

            # q projection -> qhT resident
            wq_t = load_w("wqT")
            xq_t = xpool.tile([P, EC, NQ], BF16, tag="x", name="xq_t")
            load_x(xq_t, "qT")
            for j in range(EC):
                ps = ppsum.tile([P, NQ], F32, tag="ps", name="ps")
                for e in range(EC):
                    nc.tensor.matmul(ps[:], wq_t[:, e, ts(j, P)], xq_t[:, e, :],
                                     start=(e == 0), stop=(e == EC - 1))
                nc.scalar.activation(qhT[:, j, :], ps[:], Relu, bias=bq_t[:, ds(j, 1)])
            load_masks(xpool)

        # ---------------- attention ----------------
        with tc.tile_pool(name="p", bufs=2) as ppool, \
             tc.tile_pool(name="kv", bufs=2) as kvpool, \
             tc.tile_pool(name="work", bufs=3) as wk, \
             tc.tile_pool(name="invd", bufs=2) as ivpool, \
             tc.tile_pool(name="sps", bufs=2, space="PSUM") as spsum, \
             tc.tile_pool(name="dps", bufs=1, space="PSUM") as dpsum, \
             tc.tile_pool(name="ops", bufs=1, space="PSUM") as opsum:

            spart = [ds(0, D), ds(D, D)]

            def load_pair(pair, eng=None):
                eng = eng or nc.sync
                khp = kvpool.tile([P, QB, NQ], BF16, tag="kh", name="khp")
                for g in range(QB):
                    eng.dma_start(
                        khp[:, g, :],
                        khT_ag[g, ds(pair * P, P), :])
                vhp = kvpool.tile([P, TC, P], BF16, tag="vh", name="vhp", bufs=3)
                for g in range(QB):
                    eng.dma_start(
                        vhp[:, ds(g * (TC // QB), TC // QB), :],
                        vh_ag[g].rearrange("(c p) d -> p c d", p=P)[:, :, ds(pair * P, P)])
                return khp, vhp

            def emit_av_prologue(prev):
                """broadcast 1/d (tiny rank-1 matmuls; inputs long ready)."""
                ib = []
                for hh in range(2):
                    b_ps = dpsum.tile([P, NQ], F32, tag=f"d{hh}", name=f"b_ps{hh}")
                    nc.tensor.matmul(b_ps[:], ones1f[:], prev[2][hh][:],
                                     start=True, stop=True)
                    ib_t = ivpool.tile([P, NQ], BF16, tag=f"ib{hh}", name=f"ib{hh}")
                    nc.vector.tensor_copy(ib_t[:], b_ps[:])
                    ib.append(ib_t)
                # both heads share one PSUM bank, split on the partition axis
                o_ps = opsum.tile([P, NQ], F32, tag="o", name="o_ps")
                return ib, o_ps

            def emit_av_block(prev, ib, o_ps, g):
                """normalize + mask2 + relu + attn@v for chunk-pair g of the
                PREVIOUS head pair (interleaved into the current pair)."""
                _, p_prev, _, vhp_prev = prev
                for hh in range(2):
                    ibb = ib[hh][:, None, :].broadcast_to([P, 2, NQ])
                    half = p_prev[2 * hh + g // 4]
                    p2 = wk.tile([P, 2, NQ], BF16, tag="p2", name="p2")
                    nc.vector.tensor_mul(p2[:], half[:, ts(g % 4, 2), :], ibb)
                    w_t = wk.tile([P, 2, NQ], BF16, tag="w", name="w_t")
                    nc.vector.tensor_add(w_t[:], p2[:], m2T[:, ts(g, 2), :])
                    nc.vector.tensor_scalar_max(w_t[:], w_t[:], 0.0)
                    for cc in range(2):
                        c = 2 * g + cc
                        nc.tensor.matmul(o_ps[ds(hh * D, D), :],
                                         vhp_prev[:, c, ds(hh * D, D)],
                                         w_t[:, cc, :],
                                         start=(c == 0), stop=(c == TC - 1),
                                         skip_group_check=True)

            def emit_av_epilogue(prev, o_ps):
                nc.vector.tensor_copy(headcat[:, prev[0], :], o_ps[:])

            def emit_d_chunks(d_ps, p_t, g):
                """ones-matmul accumulation of chunks 2g, 2g+1 for both heads."""
                for hh in range(2):
                    for cc in range(2):
                        c = 2 * g + cc
                        half = p_t[2 * hh + g // 4]
                        nc.tensor.matmul(d_ps[hh][:], ones128[:],
                                         half[:, (g % 4) * 2 + cc, :],
                                         start=(c == 0), stop=(c == TC - 1))

            def emit_bounce(d_ps):
                """PSUM d (both heads) -> reciprocal -> [1, NQ] 1/d each, via
                one merged DRAM reshape round-trip."""
                d_sb = ivpool.tile([1, 2 * NQ], F32, tag="dsb", name="dsb")
                for hh in range(2):
                    nc.vector.tensor_copy(d_sb[:, ds(hh * NQ, NQ)], d_ps[hh][:])
                d_dram = dbounce.tile([2 * NQ], F32, tag="dd", name="dd")
                nc.gpsimd.dma_start(d_dram[:].rearrange("(o q) -> o q", o=1), d_sb[:])
                d_r = ivpool.tile([P, 2 * NQ // P], F32, tag="dr", name="dr")
                nc.gpsimd.dma_start(d_r[:], d_dram[:].rearrange("(p f) -> p f", p=P))
                iv_r = ivpool.tile([P, 2 * NQ // P], F32, tag="ivr", name="ivr")
                nc.vector.reciprocal(iv_r[:], d_r[:])
                iv_dram = dbounce.tile([2 * NQ], F32, tag="ivd", name="ivd")
                nc.gpsimd.dma_start(iv_dram[:].rearrange("(p f) -> p f", p=P), iv_r[:])
                iv = []
                for hh in range(2):
                    iv_f = ivpool.tile([1, NQ], F32, tag=f"ivf{hh}", name=f"ivf{hh}")
                    nc.gpsimd.dma_start(
                        iv_f[:],
                        iv_dram[ds(hh * NQ, NQ)].rearrange("(o q) -> o q", o=1))
                    iv.append(iv_f)
                return iv

            prev = None
            nextkv = load_pair(0, eng=nc.gpsimd)
            for pair in range(H // 2):               # two heads per 128-row block
                khp, vhp = nextkv
                if pair + 1 < H // 2:
                    nextkv = load_pair(pair + 1)
                # p split into half-pair tiles so pair N+1's scores don't wait
                # on the full consumption of pair N-1's p
                p_t = [ppool.tile([P, TC // 2, NQ], BF16, tag=f"p{hh}{ab}",
                                  name=f"p{hh}{ab}")
                       for hh in range(2) for ab in range(2)]
                d_ps = [dpsum.tile([1, NQ], F32, tag=f"d{hh}", name=f"d_ps{hh}")
                        for hh in range(2)]
                # chunk-interleaved emission: the PE queue alternates between
                # scores (gated by exp recycling s_ps), the d ones-matmuls
                # (gated by exp one chunk back), and the previous pair's
                # attn@v (inputs all ready) - so it never stalls.
                for g in range(TC // 2):
                    for hh in range(2):
                        s_ps = spsum.tile([P, 2, NQ], F32, tag="s", name="s_ps")
                        for cc in range(2):
                            c = 2 * g + cc
                            nc.tensor.matmul(
                                s_ps[:, cc, :],
                                khp[spart[hh], c // QB, ds((c % QB) * P, P)],
                                qhT[spart[hh], pair, :], start=True, stop=True)
                        half = p_t[2 * hh + g // 4]
                        psl = half[:, ts(g % 4, 2), :]
                        nc.scalar.activation(psl, s_ps[:], Exp, scale=SCALE)
                        nc.vector.tensor_mul(psl, psl, eM[:, ts(g, 2), :])
                    if g == 2 and prev is not None:
                        ib, o_ps = emit_av_prologue(prev)
                    if g >= 1:
                        emit_d_chunks(d_ps, p_t, g - 1)
                    if g >= 2 and prev is not None:
                        emit_av_block(prev, ib, o_ps, g - 2)
                emit_d_chunks(d_ps, p_t, TC // 2 - 1)
                iv = emit_bounce(d_ps)
                if prev is not None:
                    emit_av_block(prev, ib, o_ps, TC // 2 - 2)
                    emit_av_block(prev, ib, o_ps, TC // 2 - 1)
                    emit_av_epilogue(prev, o_ps)
                prev = (pair, p_t, iv, vhp)
            ib, o_ps = emit_av_prologue(prev)
            for g in range(TC // 2):
                emit_av_block(prev, ib, o_ps, g)
            emit_av_epilogue(prev, o_ps)

        # ---------------- output projection ----------------
        with tc.tile_pool(name="wo", bufs=1) as wopool, \
             tc.tile_pool(name="ops2", bufs=4, space="PSUM") as opsum2, \
             tc.tile_pool(name="oout", bufs=4) as oopool:
            wo_t = wopool.tile([P, EC, E], BF16)
            nc.sync.dma_start(wo_t[:], io["woT"].rearrange("(eo p) d -> p eo d", p=P))
            for j in range(EC):
                ps = opsum2.tile([P, NQ], F32, tag="ps", name="ps")
                for e in range(EC):
                    nc.tensor.matmul(ps[:], wo_t[:, e, ts(j, P)], headcat[:, e, :],
                                     start=(e == 0), stop=(e == EC - 1))
                o_t = oopool.tile([P, NQ], F32, tag="o", name="o_t")
                nc.scalar.activation(o_t[:], ps[:], Relu, bias=bo_t[:, ds(j, 1)])
                nc.sync.dma_start(
                    io["outT"].rearrange("(jo p) q -> p jo q", p=P)[:, j, :], o_t[:])


_PROGRAM = None


def _build_program():
    global _PROGRAM
    if _PROGRAM is not None:
        return _PROGRAM
    nc = bacc.Bacc("TRN2", target_bir_lowering=False, debug=False,
                   num_devices=NCORES)
    io = {}
    def inp(name, shape, dt=BF16):
        io[name] = nc.dram_tensor(name, shape, dt, kind="ExternalInput").ap()
    inp("qT", [E, NQ])
    inp("kT", [E, NQ])
    inp("vT", [E, NQ])
    inp("maskT", [S, NQ])
    inp("mask2T", [S, NQ])
    for w in ("wqT", "wkT", "wvT", "woT"):
        inp(w, [E, E])
    for b in ("bq", "bk", "bo"):
        inp(b, [E], F32)
    inp("bv", [E], BF16)
    io["outT"] = nc.dram_tensor("outT", [E, NQ], F32, kind="ExternalOutput").ap()

    with tile.TileContext(nc) as tc:
        _emit(tc, io)
    nc.compile()
    _PROGRAM = (nc, io)
    return _PROGRAM


def kernel(q, k, v, mask, mask2, Wq, bq, Wk, bk, Wv, bv, Wo, bo, _trace=False):
    nc, _ = _build_program()

    def bf(x):
        return np.ascontiguousarray(x, dtype=NPBF)

    wqT = bf(Wq.T)
    wkT = bf(Wk.T)
    wvT = bf(Wv.T)
    woT = bf(Wo.T)

    in_maps = []
    for c in range(NCORES):
        b, qb = divmod(c, QB)
        rows = slice(qb * NQ, (qb + 1) * NQ)
        in_maps.append({
            "qT": bf(q[b, rows, :].T),
            "kT": bf(k[b, rows, :].T),
            "vT": bf(v[b, rows, :].T),
            "maskT": bf(mask[b, rows, :].T),
            "mask2T": bf(mask2[b, rows, :].T),
            "wqT": wqT, "wkT": wkT, "wvT": wvT, "woT": woT,
            "bq": np.ascontiguousarray(bq, dtype=np.float32),
            "bk": np.ascontiguousarray(bk, dtype=np.float32),
            "bo": np.ascontiguousarray(bo, dtype=np.float32),
            "bv": bf(bv),
        })

    res = run_bass_kernel_spmd(nc, in_maps, core_ids=list(range(NCORES)),
                               trace=_trace)

    out = np.empty((B, S, E), dtype=np.float32)
    for c in range(NCORES):
        b, qb = divmod(c, QB)
        out[b, qb * NQ:(qb + 1) * NQ, :] = res.results[c]["outT"].T
    if _trace:
        kernel.last_results = res
    return out


# revision 27
# speedup vs baseline: 1.3936x; 1.0059x over previous
"""Trainium2 Bass kernel for the masked-relu multi-head attention module.

Math (per batch b):
    qh = relu(q @ Wq.T + bq); kh, vh likewise
    scores = (qh/sqrt(D)) @ kh.T + mask        [per head]
    attn   = relu(softmax(scores) + mask2)
    out    = relu((attn @ vh)_concat @ Wo.T + bo)

Sharding: 8 cores = (batch b in 0..1) x (query block qb in 0..3).
Each core handles 512 queries of one batch, all 16 heads, all 2048 keys.
Each core projects kh/vh only for its OWN 512 tokens; the full khT/vh are
assembled with an AllGather over the 4-core batch group.

Device-side layout: scores are computed TRANSPOSED, [keys_part,
queries_free], which makes both attention matmuls transpose-free:
  scoresT = khT_chunk-as-lhsT @ qhT          (both [dim, token] layouts)
  outT    = vh-as-lhsT @ attn_T              (vh natural [token, dim])
The additive score mask becomes a multiplicative exp(mask) (computed once
per core, reused by all 16 heads); the softmax denominator (a
partition-axis sum in this layout) comes from a ones-vector matmul on the
PE, reshaped through a small DRAM bounce for the reciprocal. The
normalize + mask2 + relu + attn@v stage of head-pair N is emitted during
pair N+1 so the PE's in-order queue never stalls on the reciprocal chain.
All host-side work is pure layout (transpose / slice / cast / concat).

Compute dtype: bf16 operands with fp32 PSUM accumulation (validated
end-to-end ~5e-3 max rel err vs the fp32 reference).
"""

import sys

sys.path.insert(0, "/opt/trn_rl_repo")

import ml_dtypes
import numpy as np

from concourse import mybir
import concourse.bass as bass
import concourse.tile as tile
from concourse import bacc
from concourse.bass import ds, ts
from concourse.bass_utils import run_bass_kernel_spmd

B, S, E, H, D = 2, 2048, 1024, 16, 64
NCORES = 8
QB = NCORES // B            # query blocks per batch
NQ = S // QB                # queries per core (512)
P = 128
EC = E // P                 # 8 e-chunks
TC = S // P                 # 16 key chunks
SCALE = 1.0 / 8.0           # 1/sqrt(D)
GROUPS = [[0, 1, 2, 3], [4, 5, 6, 7]]

F32 = mybir.dt.float32
BF16 = mybir.dt.bfloat16
NPBF = ml_dtypes.bfloat16


def _emit(tc, io):
    """Emit the per-core program. io: dict of DRAM APs."""
    from contextlib import ExitStack

    nc = tc.nc
    Relu = mybir.ActivationFunctionType.Relu
    Exp = mybir.ActivationFunctionType.Exp

    with ExitStack() as ctx:
        # ---------------- constants ----------------
        cpool = ctx.enter_context(tc.tile_pool(name="const", bufs=1))
        ones128 = cpool.tile([P, 1], BF16)
        nc.vector.memset(ones128[:], 1.0)
        ones1b = cpool.tile([1, P], BF16)
        nc.vector.memset(ones1b[:], 1.0)
        ones1f = cpool.tile([1, P], F32)
        nc.vector.memset(ones1f[:], 1.0)

        bq_t = cpool.tile([P, EC], F32)
        nc.sync.dma_start(bq_t[:], io["bq"].rearrange("(j p) -> p j", p=P))
        bk_t = cpool.tile([P, EC], F32)
        nc.sync.dma_start(bk_t[:], io["bk"].rearrange("(j p) -> p j", p=P))
        bo_t = cpool.tile([P, EC], F32)
        nc.sync.dma_start(bo_t[:], io["bo"].rearrange("(j p) -> p j", p=P))
        bv_t = cpool.tile([1, E], BF16)
        nc.sync.dma_start(bv_t[:], io["bv"].rearrange("(o e) -> o e", o=1))

        # long-lived activations (all bf16)
        rpool = ctx.enter_context(tc.tile_pool(name="resident", bufs=1))
        qhT = rpool.tile([P, EC, NQ], BF16)          # [dim, q]       8 KB/par
        headcat = rpool.tile([P, EC, NQ], BF16)      # [dim, q]       8 KB/par
        eM = rpool.tile([P, TC, NQ], BF16)           # exp(maskT)    16 KB/par
        m2T = rpool.tile([P, TC, NQ], BF16)          # mask2T        16 KB/par

        dram = ctx.enter_context(tc.tile_pool(name="dram", bufs=1, space="DRAM"))
        dbounce = ctx.enter_context(tc.tile_pool(name="dbounce", bufs=2, space="DRAM"))

        # all input loads have no deps; they stream on the sync queue and are
        # ordered by first use (weights/x first - emitted in the proj block)
        def load_masks(mlp, gs):
            for g in gs:
                mt = mlp.tile([P, 2, NQ], BF16, tag="mt", name="mt")
                nc.scalar.dma_start(
                    mt[:], io["maskT"].rearrange("(c p) q -> p c q", p=P)[:, ts(g, 2), :])
                nc.scalar.activation(eM[:, ts(g, 2), :], mt[:], Exp)

        def load_m2():
            for g in range(TC // 2):
                nc.scalar.dma_start(
                    m2T[:, ts(g, 2), :],
                    io["mask2T"].rearrange("(c p) q -> p c q", p=P)[:, ts(g, 2), :])

        # ---------------- projections (own 512 tokens only) ----------------
        khT_part = dram.tile([E, NQ], BF16)          # this core's khT slice
        vh_part = dram.tile([NQ, E], BF16)           # this core's vh slice
        khT_ag = dram.tile([QB, E, NQ], BF16)
        vh_ag = dram.tile([QB, NQ, E], BF16)

        kvpool = ctx.enter_context(tc.tile_pool(name="kv", bufs=2))

        def load_khp(pair, eng):
            khp = kvpool.tile([P, QB, NQ], BF16, tag="kh", name="khp")
            for g in range(QB):
                eng.dma_start(khp[:, g, :], khT_ag[g, ds(pair * P, P), :])
            return khp

        def load_vhp(pair, eng):
            vhp = kvpool.tile([P, TC, P], BF16, tag="vh", name="vhp", bufs=3)
            for g in range(QB):
                eng.dma_start(
                    vhp[:, ds(g * (TC // QB), TC // QB), :],
                    vh_ag[g].rearrange("(c p) d -> p c d", p=P)[:, :, ds(pair * P, P)])
            return vhp

        with tc.tile_pool(name="wt", bufs=2) as wpool, \
             tc.tile_pool(name="xt", bufs=2) as xpool, \
             tc.tile_pool(name="pout", bufs=2) as opool, \
             tc.tile_pool(name="pps", bufs=4, space="PSUM") as ppsum:

            def load_w(name):
                w_t = wpool.tile([P, EC, E], BF16, tag="w", name="w_t")
                for e in range(EC):
                    nc.sync.dma_start(
                        w_t[:, e, :],
                        io[name].rearrange("(eo p) d -> p eo d", p=P)[:, e, :])
                return w_t

            def load_x(dst, name):
                for e in range(EC):
                    nc.sync.dma_start(
                        dst[:, e, :],
                        io[name].rearrange("(eo p) t -> p eo t", p=P)[:, e, :])

            # k projection -> khT_part, then AllGather early
            wk_t = load_w("wkT")
            xk_t = xpool.tile([P, EC, NQ], BF16, tag="x", name="xk_t")
            load_x(xk_t, "kT")
            kp = opool.tile([P, EC, NQ], BF16, tag="kp", name="kp")
            for j in range(EC):
                ps = ppsum.tile([P, NQ], F32, tag="ps", name="ps")
                for e in range(EC):
                    nc.tensor.matmul(ps[:], wk_t[:, e, ts(j, P)], xk_t[:, e, :],
                                     start=(e == 0), stop=(e == EC - 1))
                nc.scalar.activation(kp[:, j, :], ps[:], Relu, bias=bk_t[:, ds(j, 1)])
                nc.gpsimd.dma_start(
                    khT_part[:].rearrange("(jo p) t -> p jo t", p=P)[:, j, :],
                    kp[:, j, :])
            nc.gpsimd.collective_compute(
                "AllGather", mybir.AluOpType.bypass, replica_groups=GROUPS,
                ins=[khT_part.opt()], outs=[khT_ag.opt()])
            khp0 = load_khp(0, nc.gpsimd)
            load_masks(xpool, range(0, TC // 4))

            # v projection -> vh_part [tokens, dim]; bias rides a rank-1
            # ones-row matmul (it is along the free axis here).
            wv_t = load_w("wvT")
            xv_t = xpool.tile([P, EC, NQ], BF16, tag="x", name="xv_t")
            load_x(xv_t, "vT")
            vp = opool.tile([P, NQ // P, E], BF16, tag="vp", name="vp")
            for tc2 in range(NQ // P):              # 4 token chunks of 128
                for n in range(E // NQ):            # 2 output-dim halves of 512
                    ps = ppsum.tile([P, NQ], F32, tag="ps", name="ps")
                    for e in range(EC):
                        nc.tensor.matmul(ps[:], xv_t[:, e, ts(tc2, P)],
                                         wv_t[:, e, ts(n, NQ)],
                                         start=(e == 0), stop=False)
                    nc.tensor.matmul(ps[:], ones1b[:], bv_t[:, ts(n, NQ)],
                                     start=False, stop=True)
                    nc.scalar.activation(vp[:, tc2, ts(n, NQ)], ps[:], Relu)
                    nc.gpsimd.dma_start(
                        vh_part[:].rearrange("(c p) d -> p c d", p=P)[:, tc2, ts(n, NQ)],
                        vp[:, tc2, ts(n, NQ)])
            nc.gpsimd.collective_compute(
                "AllGather", mybir.AluOpType.bypass, replica_groups=GROUPS,
                ins=[vh_part.opt()], outs=[vh_ag.opt()])
            vhp0 = load_vhp(0, nc.gpsimd)
            load_masks(xpool, range(TC // 4, TC // 2))

            # q projection -> qhT resident
            wq_t = load_w("wqT")
            xq_t = xpool.tile([P, EC, NQ], BF16, tag="x", name="xq_t")
            load_x(xq_t, "qT")
            for j in range(EC):
                ps = ppsum.tile([P, NQ], F32, tag="ps", name="ps")
                for e in range(EC):
                    nc.tensor.matmul(ps[:], wq_t[:, e, ts(j, P)], xq_t[:, e, :],
                                     start=(e == 0), stop=(e == EC - 1))
                nc.scalar.activation(qhT[:, j, :], ps[:], Relu, bias=bq_t[:, ds(j, 1)])
            load_m2()

        # ---------------- attention ----------------
        with tc.tile_pool(name="p", bufs=2) as ppool, \
             tc.tile_pool(name="work", bufs=3) as wk, \
             tc.tile_pool(name="invd", bufs=2) as ivpool, \
             tc.tile_pool(name="sps", bufs=2, space="PSUM") as spsum, \
             tc.tile_pool(name="dps", bufs=1, space="PSUM") as dpsum, \
             tc.tile_pool(name="ops", bufs=1, space="PSUM") as opsum:

            spart = [ds(0, D), ds(D, D)]

            def emit_av_prologue(prev):
                """broadcast 1/d (tiny rank-1 matmuls; inputs long ready)."""
                ib = []
                for hh in range(2):
                    b_ps = dpsum.tile([P, NQ], F32, tag=f"d{hh}", name=f"b_ps{hh}")
                    nc.tensor.matmul(b_ps[:], ones1f[:], prev[2][hh][:],
                                     start=True, stop=True)
                    ib_t = ivpool.tile([P, NQ], BF16, tag=f"ib{hh}", name=f"ib{hh}")
                    nc.vector.tensor_copy(ib_t[:], b_ps[:])
                    ib.append(ib_t)
                # both heads share one PSUM bank, split on the partition axis
                o_ps = opsum.tile([P, NQ], F32, tag="o", name="o_ps")
                return ib, o_ps

            def emit_av_block(prev, ib, o_ps, g):
                """normalize + mask2 + relu + attn@v for chunk-pair g of the
                PREVIOUS head pair (interleaved into the current pair)."""
                _, p_prev, _, vhp_prev = prev
                for hh in range(2):
                    ibb = ib[hh][:, None, :].broadcast_to([P, 2, NQ])
                    half = p_prev[2 * hh + g // 4]
                    p2 = wk.tile([P, 2, NQ], BF16, tag="p2", name="p2")
                    nc.vector.tensor_mul(p2[:], half[:, ts(g % 4, 2), :], ibb)
                    w_t = wk.tile([P, 2, NQ], BF16, tag="w", name="w_t")
                    nc.vector.tensor_add(w_t[:], p2[:], m2T[:, ts(g, 2), :])
                    nc.vector.tensor_scalar_max(w_t[:], w_t[:], 0.0)
                    for cc in range(2):
                        c = 2 * g + cc
                        nc.tensor.matmul(o_ps[ds(hh * D, D), :],
                                         vhp_prev[:, c, ds(hh * D, D)],
                                         w_t[:, cc, :],
                                         start=(c == 0), stop=(c == TC - 1),
                                         skip_group_check=True)

            def emit_av_epilogue(prev, o_ps):
                nc.vector.tensor_copy(headcat[:, prev[0], :], o_ps[:])

            def emit_d_chunks(d_ps, p_t, g):
                """ones-matmul accumulation of chunks 2g, 2g+1 for both heads."""
                for hh in range(2):
                    for cc in range(2):
                        c = 2 * g + cc
                        half = p_t[2 * hh + g // 4]
                        nc.tensor.matmul(d_ps[hh][:], ones128[:],
                                         half[:, (g % 4) * 2 + cc, :],
                                         start=(c == 0), stop=(c == TC - 1))

            def emit_bounce(d_ps):
                """PSUM d (both heads) -> reciprocal -> [1, NQ] 1/d each, via
                one merged DRAM reshape round-trip."""
                d_sb = ivpool.tile([1, 2 * NQ], F32, tag="dsb", name="dsb")
                for hh in range(2):
                    nc.vector.tensor_copy(d_sb[:, ds(hh * NQ, NQ)], d_ps[hh][:])
                d_dram = dbounce.tile([2 * NQ], F32, tag="dd", name="dd")
                nc.gpsimd.dma_start(d_dram[:].rearrange("(o q) -> o q", o=1), d_sb[:])
                d_r = ivpool.tile([P, 2 * NQ // P], F32, tag="dr", name="dr")
                nc.gpsimd.dma_start(d_r[:], d_dram[:].rearrange("(p f) -> p f", p=P))
                iv_r = ivpool.tile([P, 2 * NQ // P], F32, tag="ivr", name="ivr")
                nc.vector.reciprocal(iv_r[:], d_r[:])
                iv_dram = dbounce.tile([2 * NQ], F32, tag="ivd", name="ivd")
                nc.gpsimd.dma_start(iv_dram[:].rearrange("(p f) -> p f", p=P), iv_r[:])
                iv = []
                for hh in range(2):
                    iv_f = ivpool.tile([1, NQ], F32, tag=f"ivf{hh}", name=f"ivf{hh}")
                    nc.gpsimd.dma_start(
                        iv_f[:],
                        iv_dram[ds(hh * NQ, NQ)].rearrange("(o q) -> o q", o=1))
                    iv.append(iv_f)
                return iv

            prev = None
            nextkv = (khp0, vhp0)
            for pair in range(H // 2):               # two heads per 128-row block
                khp, vhp = nextkv
                if pair + 1 < H // 2:
                    nextkv = (load_khp(pair + 1, nc.sync),
                              load_vhp(pair + 1, nc.sync))
                # p split into half-pair tiles so pair N+1's scores don't wait
                # on the full consumption of pair N-1's p
                p_t = [ppool.tile([P, TC // 2, NQ], BF16, tag=f"p{hh}{ab}",
                                  name=f"p{hh}{ab}")
                       for hh in range(2) for ab in range(2)]
                d_ps = [dpsum.tile([1, NQ], F32, tag=f"d{hh}", name=f"d_ps{hh}")
                        for hh in range(2)]
                # chunk-interleaved emission: the PE queue alternates between
                # scores (gated by exp recycling s_ps), the d ones-matmuls
                # (gated by exp one chunk back), and the previous pair's
                # attn@v (inputs all ready) - so it never stalls.
                for g in range(TC // 2):
                    for hh in range(2):
                        s_ps = spsum.tile([P, 2, NQ], F32, tag="s", name="s_ps")
                        for cc in range(2):
                            c = 2 * g + cc
                            nc.tensor.matmul(
                                s_ps[:, cc, :],
                                khp[spart[hh], c // QB, ds((c % QB) * P, P)],
                                qhT[spart[hh], pair, :], start=True, stop=True)
                        half = p_t[2 * hh + g // 4]
                        psl = half[:, ts(g % 4, 2), :]
                        nc.scalar.activation(psl, s_ps[:], Exp, scale=SCALE)
                        nc.vector.tensor_mul(psl, psl, eM[:, ts(g, 2), :])
                    if g == 2 and prev is not None:
                        ib, o_ps = emit_av_prologue(prev)
                    if g >= 1:
                        emit_d_chunks(d_ps, p_t, g - 1)
                    if g >= 2 and prev is not None:
                        emit_av_block(prev, ib, o_ps, g - 2)
                emit_d_chunks(d_ps, p_t, TC // 2 - 1)
                iv = emit_bounce(d_ps)
                if prev is not None:
                    emit_av_block(prev, ib, o_ps, TC // 2 - 2)
                    emit_av_block(prev, ib, o_ps, TC // 2 - 1)
                    emit_av_epilogue(prev, o_ps)
                prev = (pair, p_t, iv, vhp)
            ib, o_ps = emit_av_prologue(prev)
            for g in range(TC // 2):
                emit_av_block(prev, ib, o_ps, g)
            emit_av_epilogue(prev, o_ps)

        # ---------------- output projection ----------------
        with tc.tile_pool(name="wo", bufs=1) as wopool, \
             tc.tile_pool(name="ops2", bufs=4, space="PSUM") as opsum2, \
             tc.tile_pool(name="oout", bufs=4) as oopool:
            wo_t = wopool.tile([P, EC, E], BF16)
            nc.sync.dma_start(wo_t[:], io["woT"].rearrange("(eo p) d -> p eo d", p=P))
            for j in range(EC):
                ps = opsum2.tile([P, NQ], F32, tag="ps", name="ps")
                for e in range(EC):
                    nc.tensor.matmul(ps[:], wo_t[:, e, ts(j, P)], headcat[:, e, :],
                                     start=(e == 0), stop=(e == EC - 1))
                o_t = oopool.tile([P, NQ], F32, tag="o", name="o_t")
                nc.scalar.activation(o_t[:], ps[:], Relu, bias=bo_t[:, ds(j, 1)])
                nc.sync.dma_start(
                    io["outT"].rearrange("(jo p) q -> p jo q", p=P)[:, j, :], o_t[:])


_PROGRAM = None


def _build_program():
    global _PROGRAM
    if _PROGRAM is not None:
        return _PROGRAM
    nc = bacc.Bacc("TRN2", target_bir_lowering=False, debug=False,
                   num_devices=NCORES)
    io = {}
    def inp(name, shape, dt=BF16):
        io[name] = nc.dram_tensor(name, shape, dt, kind="ExternalInput").ap()
    inp("qT", [E, NQ])
    inp("kT", [E, NQ])
    inp("vT", [E, NQ])
    inp("maskT", [S, NQ])
    inp("mask2T", [S, NQ])
    for w in ("wqT", "wkT", "wvT", "woT"):
        inp(w, [E, E])
    for b in ("bq", "bk", "bo"):
        inp(b, [E], F32)
    inp("bv", [E], BF16)
    io["outT"] = nc.dram_tensor("outT", [E, NQ], F32, kind="ExternalOutput").ap()

    with tile.TileContext(nc) as tc:
        _emit(tc, io)
    nc.compile()
    _PROGRAM = (nc, io)
    return _PROGRAM


def kernel(q, k, v, mask, mask2, Wq, bq, Wk, bk, Wv, bv, Wo, bo, _trace=False):
    nc, _ = _build_program()

    def bf(x):
        return np.ascontiguousarray(x, dtype=NPBF)

    wqT = bf(Wq.T)
    wkT = bf(Wk.T)
    wvT = bf(Wv.T)
    woT = bf(Wo.T)

    in_maps = []
    for c in range(NCORES):
        b, qb = divmod(c, QB)
        rows = slice(qb * NQ, (qb + 1) * NQ)
        in_maps.append({
            "qT": bf(q[b, rows, :].T),
            "kT": bf(k[b, rows, :].T),
            "vT": bf(v[b, rows, :].T),
            "maskT": bf(mask[b, rows, :].T),
            "mask2T": bf(mask2[b, rows, :].T),
            "wqT": wqT, "wkT": wkT, "wvT": wvT, "woT": woT,
            "bq": np.ascontiguousarray(bq, dtype=np.float32),
            "bk": np.ascontiguousarray(bk, dtype=np.float32),
            "bo": np.ascontiguousarray(bo, dtype=np.float32),
            "bv": bf(bv),
        })

    res = run_bass_kernel_spmd(nc, in_maps, core_ids=list(range(NCORES)),
                               trace=_trace)

    out = np.empty((B, S, E), dtype=np.float32)
    for c in range(NCORES):
        b, qb = divmod(c, QB)
        out[b, qb * NQ:(qb + 1) * NQ, :] = res.results[c]["outT"].T
    if _trace:
        kernel.last_results = res
    return out


# revision 28
# speedup vs baseline: 1.3974x; 1.0027x over previous
"""Trainium2 Bass kernel for the masked-relu multi-head attention module.

Math (per batch b):
    qh = relu(q @ Wq.T + bq); kh, vh likewise
    scores = (qh/sqrt(D)) @ kh.T + mask        [per head]
    attn   = relu(softmax(scores) + mask2)
    out    = relu((attn @ vh)_concat @ Wo.T + bo)

Sharding: 8 cores = (batch b in 0..1) x (query block qb in 0..3).
Each core handles 512 queries of one batch, all 16 heads, all 2048 keys.
Each core projects kh/vh only for its OWN 512 tokens; the full khT/vh are
assembled with an AllGather over the 4-core batch group.

Device-side layout: scores are computed TRANSPOSED, [keys_part,
queries_free], which makes both attention matmuls transpose-free:
  scoresT = khT_chunk-as-lhsT @ qhT          (both [dim, token] layouts)
  outT    = vh-as-lhsT @ attn_T              (vh natural [token, dim])
The additive score mask becomes a multiplicative exp(mask) (computed once
per core, reused by all 16 heads); the softmax denominator (a
partition-axis sum in this layout) comes from a ones-vector matmul on the
PE, reshaped through a small DRAM bounce for the reciprocal. The
normalize + mask2 + relu + attn@v stage of head-pair N is emitted during
pair N+1 so the PE's in-order queue never stalls on the reciprocal chain.
All host-side work is pure layout (transpose / slice / cast / concat).

Compute dtype: bf16 operands with fp32 PSUM accumulation (validated
end-to-end ~5e-3 max rel err vs the fp32 reference).
"""

import sys

sys.path.insert(0, "/opt/trn_rl_repo")

import ml_dtypes
import numpy as np

from concourse import mybir
import concourse.bass as bass
import concourse.tile as tile
from concourse import bacc
from concourse.bass import ds, ts
from concourse.bass_utils import run_bass_kernel_spmd

B, S, E, H, D = 2, 2048, 1024, 16, 64
NCORES = 8
QB = NCORES // B            # query blocks per batch
NQ = S // QB                # queries per core (512)
P = 128
EC = E // P                 # 8 e-chunks
TC = S // P                 # 16 key chunks
SCALE = 1.0 / 8.0           # 1/sqrt(D)
GROUPS = [[0, 1, 2, 3], [4, 5, 6, 7]]

F32 = mybir.dt.float32
BF16 = mybir.dt.bfloat16
NPBF = ml_dtypes.bfloat16


def _emit(tc, io):
    """Emit the per-core program. io: dict of DRAM APs."""
    from contextlib import ExitStack

    nc = tc.nc
    Relu = mybir.ActivationFunctionType.Relu
    Exp = mybir.ActivationFunctionType.Exp

    with ExitStack() as ctx:
        # ---------------- constants ----------------
        cpool = ctx.enter_context(tc.tile_pool(name="const", bufs=1))
        ones128 = cpool.tile([P, 1], BF16)
        nc.vector.memset(ones128[:], 1.0)
        ones1b = cpool.tile([1, P], BF16)
        nc.vector.memset(ones1b[:], 1.0)
        ones1f = cpool.tile([1, P], F32)
        nc.vector.memset(ones1f[:], 1.0)

        bq_t = cpool.tile([P, EC], F32)
        nc.sync.dma_start(bq_t[:], io["bq"].rearrange("(j p) -> p j", p=P))
        bk_t = cpool.tile([P, EC], F32)
        nc.sync.dma_start(bk_t[:], io["bk"].rearrange("(j p) -> p j", p=P))
        bo_t = cpool.tile([P, EC], F32)
        nc.sync.dma_start(bo_t[:], io["bo"].rearrange("(j p) -> p j", p=P))
        bv_t = cpool.tile([1, E], BF16)
        nc.sync.dma_start(bv_t[:], io["bv"].rearrange("(o e) -> o e", o=1))

        # long-lived activations (all bf16)
        rpool = ctx.enter_context(tc.tile_pool(name="resident", bufs=1))
        qhT = rpool.tile([P, EC, NQ], BF16)          # [dim, q]       8 KB/par
        headcat = rpool.tile([P, EC, NQ], BF16)      # [dim, q]       8 KB/par
        eM = rpool.tile([P, TC, NQ], BF16)           # exp(maskT)    16 KB/par
        m2T = rpool.tile([P, TC, NQ], BF16)          # mask2T        16 KB/par

        dram = ctx.enter_context(tc.tile_pool(name="dram", bufs=1, space="DRAM"))
        dbounce = ctx.enter_context(tc.tile_pool(name="dbounce", bufs=2, space="DRAM"))

        # all input loads have no deps; they stream on the sync queue and are
        # ordered by first use (weights/x first - emitted in the proj block)
        def load_masks(mlp, gs):
            for g in gs:
                mt = mlp.tile([P, 2, NQ], BF16, tag="mt", name="mt")
                nc.scalar.dma_start(
                    mt[:], io["maskT"].rearrange("(c p) q -> p c q", p=P)[:, ts(g, 2), :])
                nc.scalar.activation(eM[:, ts(g, 2), :], mt[:], Exp)

        def load_m2():
            for g in range(TC // 2):
                nc.scalar.dma_start(
                    m2T[:, ts(g, 2), :],
                    io["mask2T"].rearrange("(c p) q -> p c q", p=P)[:, ts(g, 2), :])

        # ---------------- projections (own 512 tokens only) ----------------
        khT_part = dram.tile([E, NQ], BF16)          # this core's khT slice
        vh_part = dram.tile([NQ, E], BF16)           # this core's vh slice
        khT_ag = dram.tile([QB, E, NQ], BF16)
        vh_ag = dram.tile([QB, NQ, E], BF16)

        kvpool = ctx.enter_context(tc.tile_pool(name="kv", bufs=2))

        def load_khp(pair, eng):
            khp = kvpool.tile([P, QB, NQ], BF16, tag="kh", name="khp")
            for g in range(QB):
                eng.dma_start(khp[:, g, :], khT_ag[g, ds(pair * P, P), :])
            return khp

        def load_vhp(pair, eng):
            vhp = kvpool.tile([P, TC, P], BF16, tag="vh", name="vhp", bufs=3)
            for g in range(QB):
                eng.dma_start(
                    vhp[:, ds(g * (TC // QB), TC // QB), :],
                    vh_ag[g].rearrange("(c p) d -> p c d", p=P)[:, :, ds(pair * P, P)])
            return vhp

        with tc.tile_pool(name="wt", bufs=2) as wpool, \
             tc.tile_pool(name="xt", bufs=2) as xpool, \
             tc.tile_pool(name="pout", bufs=2) as opool, \
             tc.tile_pool(name="pps", bufs=4, space="PSUM") as ppsum:

            def load_w(name):
                w_t = wpool.tile([P, EC, E], BF16, tag="w", name="w_t")
                for e in range(EC):
                    nc.sync.dma_start(
                        w_t[:, e, :],
                        io[name].rearrange("(eo p) d -> p eo d", p=P)[:, e, :])
                return w_t

            def load_x(dst, name):
                for e in range(EC):
                    nc.sync.dma_start(
                        dst[:, e, :],
                        io[name].rearrange("(eo p) t -> p eo t", p=P)[:, e, :])

            # k projection -> khT_part, then AllGather early
            wk_t = load_w("wkT")
            xk_t = xpool.tile([P, EC, NQ], BF16, tag="x", name="xk_t")
            load_x(xk_t, "kT")
            kp = opool.tile([P, EC, NQ], BF16, tag="kp", name="kp")
            for j in range(EC):
                ps = ppsum.tile([P, NQ], F32, tag="ps", name="ps")
                for e in range(EC):
                    nc.tensor.matmul(ps[:], wk_t[:, e, ts(j, P)], xk_t[:, e, :],
                                     start=(e == 0), stop=(e == EC - 1))
                nc.scalar.activation(kp[:, j, :], ps[:], Relu, bias=bk_t[:, ds(j, 1)])
                nc.gpsimd.dma_start(
                    khT_part[:].rearrange("(jo p) t -> p jo t", p=P)[:, j, :],
                    kp[:, j, :])
            nc.gpsimd.collective_compute(
                "AllGather", mybir.AluOpType.bypass, replica_groups=GROUPS,
                ins=[khT_part.opt()], outs=[khT_ag.opt()])
            khp0 = load_khp(0, nc.gpsimd)
            load_masks(xpool, range(0, TC // 4))

            # v projection -> vh_part [tokens, dim]; bias rides a rank-1
            # ones-row matmul (it is along the free axis here).
            wv_t = load_w("wvT")
            xv_t = xpool.tile([P, EC, NQ], BF16, tag="x", name="xv_t")
            load_x(xv_t, "vT")
            vp = opool.tile([P, NQ // P, E], BF16, tag="vp", name="vp")
            for tc2 in range(NQ // P):              # 4 token chunks of 128
                for n in range(E // NQ):            # 2 output-dim halves of 512
                    ps = ppsum.tile([P, NQ], F32, tag="ps", name="ps")
                    for e in range(EC):
                        nc.tensor.matmul(ps[:], xv_t[:, e, ts(tc2, P)],
                                         wv_t[:, e, ts(n, NQ)],
                                         start=(e == 0), stop=False)
                    nc.tensor.matmul(ps[:], ones1b[:], bv_t[:, ts(n, NQ)],
                                     start=False, stop=True)
                    nc.scalar.activation(vp[:, tc2, ts(n, NQ)], ps[:], Relu)
                    nc.gpsimd.dma_start(
                        vh_part[:].rearrange("(c p) d -> p c d", p=P)[:, tc2, ts(n, NQ)],
                        vp[:, tc2, ts(n, NQ)])
            nc.gpsimd.collective_compute(
                "AllGather", mybir.AluOpType.bypass, replica_groups=GROUPS,
                ins=[vh_part.opt()], outs=[vh_ag.opt()])
            vhp0 = load_vhp(0, nc.gpsimd)
            load_masks(xpool, range(TC // 4, TC // 2))

            # q projection -> qhT resident
            wq_t = load_w("wqT")
            xq_t = xpool.tile([P, EC, NQ], BF16, tag="x", name="xq_t")
            load_x(xq_t, "qT")
            for j in range(EC):
                ps = ppsum.tile([P, NQ], F32, tag="ps", name="ps")
                for e in range(EC):
                    nc.tensor.matmul(ps[:], wq_t[:, e, ts(j, P)], xq_t[:, e, :],
                                     start=(e == 0), stop=(e == EC - 1))
                nc.scalar.activation(qhT[:, j, :], ps[:], Relu, bias=bq_t[:, ds(j, 1)])
            load_m2()
            # keep the PE (and its HAM clock) warm while the k-AllGather
            # finishes - it has no other work until khp(0) lands
            heat = ppsum.tile([1, NQ], F32, tag="heat", name="heat")
            for _ in range(140):
                nc.tensor.matmul(heat[:], ones128[:], kp[:, 0, :],
                                 start=True, stop=True, skip_group_check=True)

        # ---------------- attention ----------------
        with tc.tile_pool(name="p", bufs=2) as ppool, \
             tc.tile_pool(name="work", bufs=3) as wk, \
             tc.tile_pool(name="invd", bufs=2) as ivpool, \
             tc.tile_pool(name="sps", bufs=2, space="PSUM") as spsum, \
             tc.tile_pool(name="dps", bufs=1, space="PSUM") as dpsum, \
             tc.tile_pool(name="ops", bufs=1, space="PSUM") as opsum:

            spart = [ds(0, D), ds(D, D)]

            def emit_av_prologue(prev):
                """broadcast 1/d (tiny rank-1 matmuls; inputs long ready)."""
                ib = []
                for hh in range(2):
                    b_ps = dpsum.tile([P, NQ], F32, tag=f"d{hh}", name=f"b_ps{hh}")
                    nc.tensor.matmul(b_ps[:], ones1f[:], prev[2][hh][:],
                                     start=True, stop=True)
                    ib_t = ivpool.tile([P, NQ], BF16, tag=f"ib{hh}", name=f"ib{hh}")
                    nc.vector.tensor_copy(ib_t[:], b_ps[:])
                    ib.append(ib_t)
                # both heads share one PSUM bank, split on the partition axis
                o_ps = opsum.tile([P, NQ], F32, tag="o", name="o_ps")
                return ib, o_ps

            def emit_av_block(prev, ib, o_ps, g):
                """normalize + mask2 + relu + attn@v for chunk-pair g of the
                PREVIOUS head pair (interleaved into the current pair)."""
                _, p_prev, _, vhp_prev = prev
                for hh in range(2):
                    ibb = ib[hh][:, None, :].broadcast_to([P, 2, NQ])
                    half = p_prev[2 * hh + g // 4]
                    p2 = wk.tile([P, 2, NQ], BF16, tag="p2", name="p2")
                    nc.vector.tensor_mul(p2[:], half[:, ts(g % 4, 2), :], ibb)
                    w_t = wk.tile([P, 2, NQ], BF16, tag="w", name="w_t")
                    nc.vector.tensor_add(w_t[:], p2[:], m2T[:, ts(g, 2), :])
                    nc.vector.tensor_scalar_max(w_t[:], w_t[:], 0.0)
                    for cc in range(2):
                        c = 2 * g + cc
                        nc.tensor.matmul(o_ps[ds(hh * D, D), :],
                                         vhp_prev[:, c, ds(hh * D, D)],
                                         w_t[:, cc, :],
                                         start=(c == 0), stop=(c == TC - 1),
                                         skip_group_check=True)

            def emit_av_epilogue(prev, o_ps):
                nc.scalar.copy(headcat[:, prev[0], :], o_ps[:])

            def emit_d_chunks(d_ps, p_t, g):
                """ones-matmul accumulation of chunks 2g, 2g+1 for both heads."""
                for hh in range(2):
                    for cc in range(2):
                        c = 2 * g + cc
                        half = p_t[2 * hh + g // 4]
                        nc.tensor.matmul(d_ps[hh][:], ones128[:],
                                         half[:, (g % 4) * 2 + cc, :],
                                         start=(c == 0), stop=(c == TC - 1))

            def emit_bounce(d_ps):
                """PSUM d (both heads) -> reciprocal -> [1, NQ] 1/d each, via
                one merged DRAM reshape round-trip."""
                d_sb = ivpool.tile([1, 2 * NQ], F32, tag="dsb", name="dsb")
                for hh in range(2):
                    nc.scalar.copy(d_sb[:, ds(hh * NQ, NQ)], d_ps[hh][:])
                d_dram = dbounce.tile([2 * NQ], F32, tag="dd", name="dd")
                nc.gpsimd.dma_start(d_dram[:].rearrange("(o q) -> o q", o=1), d_sb[:])
                d_r = ivpool.tile([P, 2 * NQ // P], F32, tag="dr", name="dr")
                nc.gpsimd.dma_start(d_r[:], d_dram[:].rearrange("(p f) -> p f", p=P))
                iv_r = ivpool.tile([P, 2 * NQ // P], F32, tag="ivr", name="ivr")
                nc.vector.reciprocal(iv_r[:], d_r[:])
                iv_dram = dbounce.tile([2 * NQ], F32, tag="ivd", name="ivd")
                nc.gpsimd.dma_start(iv_dram[:].rearrange("(p f) -> p f", p=P), iv_r[:])
                iv = []
                for hh in range(2):
                    iv_f = ivpool.tile([1, NQ], F32, tag=f"ivf{hh}", name=f"ivf{hh}")
                    nc.gpsimd.dma_start(
                        iv_f[:],
                        iv_dram[ds(hh * NQ, NQ)].rearrange("(o q) -> o q", o=1))
                    iv.append(iv_f)
                return iv

            prev = None
            nextkv = (khp0, vhp0)
            for pair in range(H // 2):               # two heads per 128-row block
                khp, vhp = nextkv
                if pair + 1 < H // 2:
                    nextkv = (load_khp(pair + 1, nc.sync),
                              load_vhp(pair + 1, nc.sync))
                # p split into half-pair tiles so pair N+1's scores don't wait
                # on the full consumption of pair N-1's p
                p_t = [ppool.tile([P, TC // 2, NQ], BF16, tag=f"p{hh}{ab}",
                                  name=f"p{hh}{ab}")
                       for hh in range(2) for ab in range(2)]
                d_ps = [dpsum.tile([1, NQ], F32, tag=f"d{hh}", name=f"d_ps{hh}")
                        for hh in range(2)]
                # chunk-interleaved emission: the PE queue alternates between
                # scores (gated by exp recycling s_ps), the d ones-matmuls
                # (gated by exp one chunk back), and the previous pair's
                # attn@v (inputs all ready) - so it never stalls.
                for g in range(TC // 2):
                    for hh in range(2):
                        s_ps = spsum.tile([P, 2, NQ], F32, tag="s", name="s_ps")
                        for cc in range(2):
                            c = 2 * g + cc
                            nc.tensor.matmul(
                                s_ps[:, cc, :],
                                khp[spart[hh], c // QB, ds((c % QB) * P, P)],
                                qhT[spart[hh], pair, :], start=True, stop=True)
                        half = p_t[2 * hh + g // 4]
                        psl = half[:, ts(g % 4, 2), :]
                        nc.scalar.activation(psl, s_ps[:], Exp, scale=SCALE)
                        nc.vector.tensor_mul(psl, psl, eM[:, ts(g, 2), :])
                    if g == 2 and prev is not None:
                        ib, o_ps = emit_av_prologue(prev)
                    if g >= 1:
                        emit_d_chunks(d_ps, p_t, g - 1)
                    if g >= 2 and prev is not None:
                        emit_av_block(prev, ib, o_ps, g - 2)
                emit_d_chunks(d_ps, p_t, TC // 2 - 1)
                iv = emit_bounce(d_ps)
                if prev is not None:
                    emit_av_block(prev, ib, o_ps, TC // 2 - 2)
                    emit_av_block(prev, ib, o_ps, TC // 2 - 1)
                    emit_av_epilogue(prev, o_ps)
                prev = (pair, p_t, iv, vhp)
            ib, o_ps = emit_av_prologue(prev)
            for g in range(TC // 2):
                emit_av_block(prev, ib, o_ps, g)
            emit_av_epilogue(prev, o_ps)

        # ---------------- output projection ----------------
        with tc.tile_pool(name="wo", bufs=1) as wopool, \
             tc.tile_pool(name="ops2", bufs=4, space="PSUM") as opsum2, \
             tc.tile_pool(name="oout", bufs=4) as oopool:
            wo_t = wopool.tile([P, EC, E], BF16)
            nc.sync.dma_start(wo_t[:], io["woT"].rearrange("(eo p) d -> p eo d", p=P))
            for j in range(EC):
                ps = opsum2.tile([P, NQ], F32, tag="ps", name="ps")
                for e in range(EC):
                    nc.tensor.matmul(ps[:], wo_t[:, e, ts(j, P)], headcat[:, e, :],
                                     start=(e == 0), stop=(e == EC - 1))
                o_t = oopool.tile([P, NQ], F32, tag="o", name="o_t")
                nc.scalar.activation(o_t[:], ps[:], Relu, bias=bo_t[:, ds(j, 1)])
                nc.sync.dma_start(
                    io["outT"].rearrange("(jo p) q -> p jo q", p=P)[:, j, :], o_t[:])


_PROGRAM = None


def _build_program():
    global _PROGRAM
    if _PROGRAM is not None:
        return _PROGRAM
    nc = bacc.Bacc("TRN2", target_bir_lowering=False, debug=False,
                   num_devices=NCORES)
    io = {}
    def inp(name, shape, dt=BF16):
        io[name] = nc.dram_tensor(name, shape, dt, kind="ExternalInput").ap()
    inp("qT", [E, NQ])
    inp("kT", [E, NQ])
    inp("vT", [E, NQ])
    inp("maskT", [S, NQ])
    inp("mask2T", [S, NQ])
    for w in ("wqT", "wkT", "wvT", "woT"):
        inp(w, [E, E])
    for b in ("bq", "bk", "bo"):
        inp(b, [E], F32)
    inp("bv", [E], BF16)
    io["outT"] = nc.dram_tensor("outT", [E, NQ], F32, kind="ExternalOutput").ap()

    with tile.TileContext(nc) as tc:
        _emit(tc, io)
    nc.compile()
    _PROGRAM = (nc, io)
    return _PROGRAM


def kernel(q, k, v, mask, mask2, Wq, bq, Wk, bk, Wv, bv, Wo, bo, _trace=False):
    nc, _ = _build_program()

    def bf(x):
        return np.ascontiguousarray(x, dtype=NPBF)

    wqT = bf(Wq.T)
    wkT = bf(Wk.T)
    wvT = bf(Wv.T)
    woT = bf(Wo.T)

    in_maps = []
    for c in range(NCORES):
        b, qb = divmod(c, QB)
        rows = slice(qb * NQ, (qb + 1) * NQ)
        in_maps.append({
            "qT": bf(q[b, rows, :].T),
            "kT": bf(k[b, rows, :].T),
            "vT": bf(v[b, rows, :].T),
            "maskT": bf(mask[b, rows, :].T),
            "mask2T": bf(mask2[b, rows, :].T),
            "wqT": wqT, "wkT": wkT, "wvT": wvT, "woT": woT,
            "bq": np.ascontiguousarray(bq, dtype=np.float32),
            "bk": np.ascontiguousarray(bk, dtype=np.float32),
            "bo": np.ascontiguousarray(bo, dtype=np.float32),
            "bv": bf(bv),
        })

    res = run_bass_kernel_spmd(nc, in_maps, core_ids=list(range(NCORES)),
                               trace=_trace)

    out = np.empty((B, S, E), dtype=np.float32)
    for c in range(NCORES):
        b, qb = divmod(c, QB)
        out[b, qb * NQ:(qb + 1) * NQ, :] = res.results[c]["outT"].T
    if _trace:
        kernel.last_results = res
    return out


# revision 29
# speedup vs baseline: 1.4553x; 1.0414x over previous
"""Trainium2 Bass kernel for the masked-relu multi-head attention module.

Math (per batch b):
    qh = relu(q @ Wq.T + bq); kh, vh likewise
    scores = (qh/sqrt(D)) @ kh.T + mask        [per head]
    attn   = relu(softmax(scores) + mask2)
    out    = relu((attn @ vh)_concat @ Wo.T + bo)

Sharding: 8 cores = (batch b in 0..1) x (query block qb in 0..3).
Each core handles 512 queries of one batch, all 16 heads, all 2048 keys.
Each core projects kh/vh only for its OWN 512 tokens; the full khT/vh are
assembled with an AllGather over the 4-core batch group.

Device-side layout: scores are computed TRANSPOSED, [keys_part,
queries_free], which makes both attention matmuls transpose-free:
  scoresT = khT_chunk-as-lhsT @ qhT          (both [dim, token] layouts)
  outT    = vh-as-lhsT @ attn_T              (vh natural [token, dim])
The additive score mask becomes a multiplicative exp(mask) (computed once
per core, reused by all 16 heads); the softmax denominator (a
partition-axis sum in this layout) comes from a ones-vector matmul on the
PE, reshaped through a small DRAM bounce for the reciprocal. The
normalize + mask2 + relu + attn@v stage of head-pair N is emitted during
pair N+1 so the PE's in-order queue never stalls on the reciprocal chain.
All host-side work is pure layout (transpose / slice / cast / concat).

Compute dtype: bf16 operands with fp32 PSUM accumulation (validated
end-to-end ~5e-3 max rel err vs the fp32 reference).
"""

import sys

sys.path.insert(0, "/opt/trn_rl_repo")

import ml_dtypes
import numpy as np

from concourse import mybir
import concourse.bass as bass
import concourse.tile as tile
from concourse import bacc
from concourse.bass import ds, ts
from concourse.bass_utils import run_bass_kernel_spmd

B, S, E, H, D = 2, 2048, 1024, 16, 64
NCORES = 8
QB = NCORES // B            # query blocks per batch
NQ = S // QB                # queries per core (512)
P = 128
EC = E // P                 # 8 e-chunks
TC = S // P                 # 16 key chunks
SCALE = 1.0 / 8.0           # 1/sqrt(D)
GROUPS = [[0, 1, 2, 3], [4, 5, 6, 7]]

F32 = mybir.dt.float32
BF16 = mybir.dt.bfloat16
NPBF = ml_dtypes.bfloat16


def _emit(tc, io):
    """Emit the per-core program. io: dict of DRAM APs."""
    from contextlib import ExitStack

    nc = tc.nc
    Relu = mybir.ActivationFunctionType.Relu
    Exp = mybir.ActivationFunctionType.Exp

    with ExitStack() as ctx:
        # ---------------- constants ----------------
        cpool = ctx.enter_context(tc.tile_pool(name="const", bufs=1))
        ones128 = cpool.tile([P, 1], BF16)
        nc.vector.memset(ones128[:], 1.0)
        ones1b = cpool.tile([1, P], BF16)
        nc.vector.memset(ones1b[:], 1.0)
        ones1f = cpool.tile([1, P], F32)
        nc.vector.memset(ones1f[:], 1.0)

        bq_t = cpool.tile([P, EC], F32)
        nc.sync.dma_start(bq_t[:], io["bq"].rearrange("(j p) -> p j", p=P))
        bk_t = cpool.tile([P, EC], F32)
        nc.sync.dma_start(bk_t[:], io["bk"].rearrange("(j p) -> p j", p=P))
        bo_t = cpool.tile([P, EC], F32)
        nc.sync.dma_start(bo_t[:], io["bo"].rearrange("(j p) -> p j", p=P))
        bv_t = cpool.tile([1, E], BF16)
        nc.sync.dma_start(bv_t[:], io["bv"].rearrange("(o e) -> o e", o=1))

        # long-lived activations (all bf16)
        rpool = ctx.enter_context(tc.tile_pool(name="resident", bufs=1))
        qhT = rpool.tile([P, EC, NQ], BF16)          # [dim, q]       8 KB/par
        headcat = rpool.tile([P, EC, NQ], BF16)      # [dim, q]       8 KB/par
        eM = rpool.tile([P, TC, NQ], BF16)           # exp(maskT)    16 KB/par
        m2T = rpool.tile([P, TC, NQ], BF16)          # mask2T        16 KB/par

        dram = ctx.enter_context(tc.tile_pool(name="dram", bufs=1, space="DRAM"))
        dbounce = ctx.enter_context(tc.tile_pool(name="dbounce", bufs=2, space="DRAM"))

        # all input loads have no deps; they stream on the sync queue and are
        # ordered by first use (weights/x first - emitted in the proj block)
        def load_masks(mlp, gs):
            for g in gs:
                mt = mlp.tile([P, 2, NQ], BF16, tag="mt", name="mt")
                nc.scalar.dma_start(
                    mt[:], io["maskT"].rearrange("(c p) q -> p c q", p=P)[:, ts(g, 2), :])
                nc.scalar.activation(eM[:, ts(g, 2), :], mt[:], Exp)

        def load_m2():
            for g in range(TC // 2):
                nc.scalar.dma_start(
                    m2T[:, ts(g, 2), :],
                    io["mask2T"].rearrange("(c p) q -> p c q", p=P)[:, ts(g, 2), :])

        # ---------------- projections (own 512 tokens only) ----------------
        khT_part = dram.tile([E, NQ], BF16)          # this core's khT slice
        vh_part = dram.tile([NQ, E], BF16)           # this core's vh slice
        khT_ag = dram.tile([QB, E, NQ], BF16)
        vh_ag = dram.tile([QB, NQ, E], BF16)

        kvpool = ctx.enter_context(tc.tile_pool(name="kv", bufs=2))

        def load_khp(pair, eng):
            khp = kvpool.tile([P, QB, NQ], BF16, tag="kh", name="khp")
            for g in range(QB):
                eng.dma_start(khp[:, g, :], khT_ag[g, ds(pair * P, P), :])
            return khp

        def load_vhp(pair, eng):
            vhp = kvpool.tile([P, TC, P], BF16, tag="vh", name="vhp", bufs=3)
            for g in range(QB):
                eng.dma_start(
                    vhp[:, ds(g * (TC // QB), TC // QB), :],
                    vh_ag[g].rearrange("(c p) d -> p c d", p=P)[:, :, ds(pair * P, P)])
            return vhp

        with tc.tile_pool(name="wt", bufs=2) as wpool, \
             tc.tile_pool(name="xt", bufs=2) as xpool, \
             tc.tile_pool(name="pout", bufs=2) as opool, \
             tc.tile_pool(name="pps", bufs=4, space="PSUM") as ppsum:

            def load_w(name):
                w_t = wpool.tile([P, EC, E], BF16, tag="w", name="w_t")
                for e in range(EC):
                    nc.sync.dma_start(
                        w_t[:, e, :],
                        io[name].rearrange("(eo p) d -> p eo d", p=P)[:, e, :])
                return w_t

            def load_x(dst, name):
                for e in range(EC):
                    nc.sync.dma_start(
                        dst[:, e, :],
                        io[name].rearrange("(eo p) t -> p eo t", p=P)[:, e, :])

            # k projection -> khT_part, then AllGather early
            wk_t = load_w("wkT")
            xk_t = xpool.tile([P, EC, NQ], BF16, tag="x", name="xk_t")
            load_x(xk_t, "kT")
            kp = opool.tile([P, EC, NQ], BF16, tag="kp", name="kp")
            for j in range(EC):
                ps = ppsum.tile([P, NQ], F32, tag="ps", name="ps")
                for e in range(EC):
                    nc.tensor.matmul(ps[:], wk_t[:, e, ts(j, P)], xk_t[:, e, :],
                                     start=(e == 0), stop=(e == EC - 1))
                nc.scalar.activation(kp[:, j, :], ps[:], Relu, bias=bk_t[:, ds(j, 1)])
                nc.gpsimd.dma_start(
                    khT_part[:].rearrange("(jo p) t -> p jo t", p=P)[:, j, :],
                    kp[:, j, :])
            nc.gpsimd.collective_compute(
                "AllGather", mybir.AluOpType.bypass, replica_groups=GROUPS,
                ins=[khT_part.opt()], outs=[khT_ag.opt()])
            khp0 = load_khp(0, nc.gpsimd)
            load_masks(xpool, range(0, TC // 4))

            # v projection -> vh_part [tokens, dim]; bias rides a rank-1
            # ones-row matmul (it is along the free axis here).
            wv_t = load_w("wvT")
            xv_t = xpool.tile([P, EC, NQ], BF16, tag="x", name="xv_t")
            load_x(xv_t, "vT")
            vp = opool.tile([P, NQ // P, E], BF16, tag="vp", name="vp")
            for tc2 in range(NQ // P):              # 4 token chunks of 128
                for n in range(E // NQ):            # 2 output-dim halves of 512
                    ps = ppsum.tile([P, NQ], F32, tag="ps", name="ps")
                    for e in range(EC):
                        nc.tensor.matmul(ps[:], xv_t[:, e, ts(tc2, P)],
                                         wv_t[:, e, ts(n, NQ)],
                                         start=(e == 0), stop=False)
                    nc.tensor.matmul(ps[:], ones1b[:], bv_t[:, ts(n, NQ)],
                                     start=False, stop=True)
                    nc.scalar.activation(vp[:, tc2, ts(n, NQ)], ps[:], Relu)
                    nc.gpsimd.dma_start(
                        vh_part[:].rearrange("(c p) d -> p c d", p=P)[:, tc2, ts(n, NQ)],
                        vp[:, tc2, ts(n, NQ)])
            nc.gpsimd.collective_compute(
                "AllGather", mybir.AluOpType.bypass, replica_groups=GROUPS,
                ins=[vh_part.opt()], outs=[vh_ag.opt()])
            vhp0 = load_vhp(0, nc.gpsimd)
            load_masks(xpool, range(TC // 4, TC // 2))

            # q projection -> qhT resident
            wq_t = load_w("wqT")
            xq_t = xpool.tile([P, EC, NQ], BF16, tag="x", name="xq_t")
            load_x(xq_t, "qT")
            for j in range(EC):
                ps = ppsum.tile([P, NQ], F32, tag="ps", name="ps")
                for e in range(EC):
                    nc.tensor.matmul(ps[:], wq_t[:, e, ts(j, P)], xq_t[:, e, :],
                                     start=(e == 0), stop=(e == EC - 1))
                nc.scalar.activation(qhT[:, j, :], ps[:], Relu, bias=bq_t[:, ds(j, 1)])
            load_m2()
            # keep the PE (and its HAM clock) warm while the k-AllGather
            # finishes - it has no other work until khp(0) lands
            heat = ppsum.tile([1, NQ], F32, tag="heat", name="heat")
            for _ in range(190):
                nc.tensor.matmul(heat[:], ones128[:], kp[:, 0, :],
                                 start=True, stop=True, skip_group_check=True)

        # ---------------- attention ----------------
        with tc.tile_pool(name="p", bufs=2) as ppool, \
             tc.tile_pool(name="work", bufs=3) as wk, \
             tc.tile_pool(name="invd", bufs=2) as ivpool, \
             tc.tile_pool(name="sps", bufs=2, space="PSUM") as spsum, \
             tc.tile_pool(name="dps", bufs=1, space="PSUM") as dpsum, \
             tc.tile_pool(name="ops", bufs=1, space="PSUM") as opsum:

            spart = [ds(0, D), ds(D, D)]

            def emit_av_prologue(prev):
                """broadcast 1/d (tiny rank-1 matmuls; inputs long ready)."""
                ib = []
                for hh in range(2):
                    b_ps = dpsum.tile([P, NQ], F32, tag=f"d{hh}", name=f"b_ps{hh}")
                    nc.tensor.matmul(b_ps[:], ones1f[:], prev[2][hh][:],
                                     start=True, stop=True)
                    ib_t = ivpool.tile([P, NQ], BF16, tag=f"ib{hh}", name=f"ib{hh}")
                    nc.scalar.copy(ib_t[:], b_ps[:])
                    ib.append(ib_t)
                # both heads share one PSUM bank, split on the partition axis
                o_ps = opsum.tile([P, NQ], F32, tag="o", name="o_ps")
                return ib, o_ps

            def emit_av_block(prev, ib, o_ps, g):
                """normalize + mask2 + relu + attn@v for chunk-pair g of the
                PREVIOUS head pair (interleaved into the current pair)."""
                _, p_prev, _, vhp_prev = prev
                for hh in range(2):
                    ibb = ib[hh][:, None, :].broadcast_to([P, 2, NQ])
                    half = p_prev[2 * hh + g // 4]
                    p2 = wk.tile([P, 2, NQ], BF16, tag="p2", name="p2")
                    nc.vector.tensor_mul(p2[:], half[:, ts(g % 4, 2), :], ibb)
                    w_t = wk.tile([P, 2, NQ], BF16, tag="w", name="w_t")
                    nc.vector.tensor_add(w_t[:], p2[:], m2T[:, ts(g, 2), :])
                    nc.vector.tensor_scalar_max(w_t[:], w_t[:], 0.0)
                    for cc in range(2):
                        c = 2 * g + cc
                        nc.tensor.matmul(o_ps[ds(hh * D, D), :],
                                         vhp_prev[:, c, ds(hh * D, D)],
                                         w_t[:, cc, :],
                                         start=(c == 0), stop=(c == TC - 1),
                                         skip_group_check=True)

            def emit_av_epilogue(prev, o_ps):
                nc.scalar.copy(headcat[:, prev[0], :], o_ps[:])

            def emit_d_chunks(d_ps, p_t, g):
                """ones-matmul accumulation of chunks 2g, 2g+1 for both heads."""
                for hh in range(2):
                    for cc in range(2):
                        c = 2 * g + cc
                        half = p_t[2 * hh + g // 4]
                        nc.tensor.matmul(d_ps[hh][:], ones128[:],
                                         half[:, (g % 4) * 2 + cc, :],
                                         start=(c == 0), stop=(c == TC - 1))

            def emit_bounce(d_ps):
                """PSUM d (both heads) -> reciprocal -> [1, NQ] 1/d each, via
                one merged DRAM reshape round-trip."""
                d_sb = ivpool.tile([1, 2 * NQ], F32, tag="dsb", name="dsb")
                for hh in range(2):
                    nc.scalar.copy(d_sb[:, ds(hh * NQ, NQ)], d_ps[hh][:])
                d_dram = dbounce.tile([2 * NQ], F32, tag="dd", name="dd")
                nc.gpsimd.dma_start(d_dram[:].rearrange("(o q) -> o q", o=1), d_sb[:])
                d_r = ivpool.tile([P, 2 * NQ // P], F32, tag="dr", name="dr")
                nc.gpsimd.dma_start(d_r[:], d_dram[:].rearrange("(p f) -> p f", p=P))
                iv_r = ivpool.tile([P, 2 * NQ // P], F32, tag="ivr", name="ivr")
                nc.vector.reciprocal(iv_r[:], d_r[:])
                iv_dram = dbounce.tile([2 * NQ], F32, tag="ivd", name="ivd")
                nc.gpsimd.dma_start(iv_dram[:].rearrange("(p f) -> p f", p=P), iv_r[:])
                iv = []
                for hh in range(2):
                    iv_f = ivpool.tile([1, NQ], F32, tag=f"ivf{hh}", name=f"ivf{hh}")
                    nc.gpsimd.dma_start(
                        iv_f[:],
                        iv_dram[ds(hh * NQ, NQ)].rearrange("(o q) -> o q", o=1))
                    iv.append(iv_f)
                return iv

            prev = None
            nextkv = (khp0, vhp0)
            for pair in range(H // 2):               # two heads per 128-row block
                khp, vhp = nextkv
                if pair + 1 < H // 2:
                    nextkv = (load_khp(pair + 1, nc.sync),
                              load_vhp(pair + 1, nc.sync))
                # p split into half-pair tiles so pair N+1's scores don't wait
                # on the full consumption of pair N-1's p
                p_t = [ppool.tile([P, TC // 2, NQ], BF16, tag=f"p{hh}{ab}",
                                  name=f"p{hh}{ab}")
                       for hh in range(2) for ab in range(2)]
                d_ps = [dpsum.tile([1, NQ], F32, tag=f"d{hh}", name=f"d_ps{hh}")
                        for hh in range(2)]
                # chunk-interleaved emission: the PE queue alternates between
                # scores (gated by exp recycling s_ps), the d ones-matmuls
                # (gated by exp one chunk back), and the previous pair's
                # attn@v (inputs all ready) - so it never stalls.
                for g in range(TC // 2):
                    for hh in range(2):
                        s_ps = spsum.tile([P, 2, NQ], F32, tag="s", name="s_ps")
                        for cc in range(2):
                            c = 2 * g + cc
                            nc.tensor.matmul(
                                s_ps[:, cc, :],
                                khp[spart[hh], c // QB, ds((c % QB) * P, P)],
                                qhT[spart[hh], pair, :], start=True, stop=True)
                        half = p_t[2 * hh + g // 4]
                        psl = half[:, ts(g % 4, 2), :]
                        nc.scalar.activation(psl, s_ps[:], Exp, scale=SCALE)
                        nc.vector.tensor_mul(psl, psl, eM[:, ts(g, 2), :])
                    if g == 1 and prev is not None:
                        ib, o_ps = emit_av_prologue(prev)
                    if g >= 1:
                        emit_d_chunks(d_ps, p_t, g - 1)
                    if g >= 2 and prev is not None:
                        emit_av_block(prev, ib, o_ps, g - 2)
                emit_d_chunks(d_ps, p_t, TC // 2 - 1)
                iv = emit_bounce(d_ps)
                if prev is not None:
                    emit_av_block(prev, ib, o_ps, TC // 2 - 2)
                    emit_av_block(prev, ib, o_ps, TC // 2 - 1)
                    emit_av_epilogue(prev, o_ps)
                prev = (pair, p_t, iv, vhp)
            ib, o_ps = emit_av_prologue(prev)
            for g in range(TC // 2):
                emit_av_block(prev, ib, o_ps, g)
            emit_av_epilogue(prev, o_ps)

        # ---------------- output projection ----------------
        with tc.tile_pool(name="wo", bufs=1) as wopool, \
             tc.tile_pool(name="ops2", bufs=4, space="PSUM") as opsum2, \
             tc.tile_pool(name="oout", bufs=4) as oopool:
            wo_t = wopool.tile([P, EC, E], BF16)
            nc.sync.dma_start(wo_t[:], io["woT"].rearrange("(eo p) d -> p eo d", p=P))
            for j in range(EC):
                ps = opsum2.tile([P, NQ], F32, tag="ps", name="ps")
                for e in range(EC):
                    nc.tensor.matmul(ps[:], wo_t[:, e, ts(j, P)], headcat[:, e, :],
                                     start=(e == 0), stop=(e == EC - 1))
                o_t = oopool.tile([P, NQ], F32, tag="o", name="o_t")
                nc.scalar.activation(o_t[:], ps[:], Relu, bias=bo_t[:, ds(j, 1)])
                nc.sync.dma_start(
                    io["outT"].rearrange("(jo p) q -> p jo q", p=P)[:, j, :], o_t[:])


_PROGRAM = None


def _build_program():
    global _PROGRAM
    if _PROGRAM is not None:
        return _PROGRAM
    nc = bacc.Bacc("TRN2", target_bir_lowering=False, debug=False,
                   num_devices=NCORES)
    io = {}
    def inp(name, shape, dt=BF16):
        io[name] = nc.dram_tensor(name, shape, dt, kind="ExternalInput").ap()
    inp("qT", [E, NQ])
    inp("kT", [E, NQ])
    inp("vT", [E, NQ])
    inp("maskT", [S, NQ])
    inp("mask2T", [S, NQ])
    for w in ("wqT", "wkT", "wvT", "woT"):
        inp(w, [E, E])
    for b in ("bq", "bk", "bo"):
        inp(b, [E], F32)
    inp("bv", [E], BF16)
    io["outT"] = nc.dram_tensor("outT", [E, NQ], F32, kind="ExternalOutput").ap()

    with tile.TileContext(nc) as tc:
        _emit(tc, io)
    nc.compile()
    _PROGRAM = (nc, io)
    return _PROGRAM


def kernel(q, k, v, mask, mask2, Wq, bq, Wk, bk, Wv, bv, Wo, bo, _trace=False):
    nc, _ = _build_program()

    def bf(x):
        return np.ascontiguousarray(x, dtype=NPBF)

    wqT = bf(Wq.T)
    wkT = bf(Wk.T)
    wvT = bf(Wv.T)
    woT = bf(Wo.T)

    in_maps = []
    for c in range(NCORES):
        b, qb = divmod(c, QB)
        rows = slice(qb * NQ, (qb + 1) * NQ)
        in_maps.append({
            "qT": bf(q[b, rows, :].T),
            "kT": bf(k[b, rows, :].T),
            "vT": bf(v[b, rows, :].T),
            "maskT": bf(mask[b, rows, :].T),
            "mask2T": bf(mask2[b, rows, :].T),
            "wqT": wqT, "wkT": wkT, "wvT": wvT, "woT": woT,
            "bq": np.ascontiguousarray(bq, dtype=np.float32),
            "bk": np.ascontiguousarray(bk, dtype=np.float32),
            "bo": np.ascontiguousarray(bo, dtype=np.float32),
            "bv": bf(bv),
        })

    res = run_bass_kernel_spmd(nc, in_maps, core_ids=list(range(NCORES)),
                               trace=_trace)

    out = np.empty((B, S, E), dtype=np.float32)
    for c in range(NCORES):
        b, qb = divmod(c, QB)
        out[b, qb * NQ:(qb + 1) * NQ, :] = res.results[c]["outT"].T
    if _trace:
        kernel.last_results = res
    return out


# revision 30
# speedup vs baseline: 1.4883x; 1.0227x over previous
"""Trainium2 Bass kernel for the masked-relu multi-head attention module.

Math (per batch b):
    qh = relu(q @ Wq.T + bq); kh, vh likewise
    scores = (qh/sqrt(D)) @ kh.T + mask        [per head]
    attn   = relu(softmax(scores) + mask2)
    out    = relu((attn @ vh)_concat @ Wo.T + bo)

Sharding: 8 cores = (batch b in 0..1) x (query block qb in 0..3).
Each core handles 512 queries of one batch, all 16 heads, all 2048 keys.
Each core projects kh/vh only for its OWN 512 tokens; the full khT/vh are
assembled with an AllGather over the 4-core batch group.

Device-side layout: scores are computed TRANSPOSED, [keys_part,
queries_free], which makes both attention matmuls transpose-free:
  scoresT = khT_chunk-as-lhsT @ qhT          (both [dim, token] layouts)
  outT    = vh-as-lhsT @ attn_T              (vh natural [token, dim])
The additive score mask becomes a multiplicative exp(mask) (computed once
per core, reused by all 16 heads); the softmax denominator (a
partition-axis sum in this layout) comes from a ones-vector matmul on the
PE, reshaped through a small DRAM bounce for the reciprocal. The
normalize + mask2 + relu + attn@v stage of head-pair N is emitted during
pair N+1 so the PE's in-order queue never stalls on the reciprocal chain.
All host-side work is pure layout (transpose / slice / cast / concat).

Compute dtype: bf16 operands with fp32 PSUM accumulation (validated
end-to-end ~5e-3 max rel err vs the fp32 reference).
"""

import sys

sys.path.insert(0, "/opt/trn_rl_repo")

import ml_dtypes
import numpy as np

from concourse import mybir
import concourse.bass as bass
import concourse.tile as tile
from concourse import bacc
from concourse.bass import ds, ts
from concourse.bass_utils import run_bass_kernel_spmd

B, S, E, H, D = 2, 2048, 1024, 16, 64
NCORES = 8
QB = NCORES // B            # query blocks per batch
NQ = S // QB                # queries per core (512)
P = 128
EC = E // P                 # 8 e-chunks
TC = S // P                 # 16 key chunks
SCALE = 1.0 / 8.0           # 1/sqrt(D)
GROUPS = [[0, 1, 2, 3], [4, 5, 6, 7]]

F32 = mybir.dt.float32
BF16 = mybir.dt.bfloat16
NPBF = ml_dtypes.bfloat16


def _emit(tc, io):
    """Emit the per-core program. io: dict of DRAM APs."""
    from contextlib import ExitStack

    nc = tc.nc
    Relu = mybir.ActivationFunctionType.Relu
    Exp = mybir.ActivationFunctionType.Exp

    with ExitStack() as ctx:
        # ---------------- constants ----------------
        cpool = ctx.enter_context(tc.tile_pool(name="const", bufs=1))
        ones128 = cpool.tile([P, 1], BF16)
        nc.vector.memset(ones128[:], 1.0)
        ones1b = cpool.tile([1, P], BF16)
        nc.vector.memset(ones1b[:], 1.0)
        ones1f = cpool.tile([1, P], F32)
        nc.vector.memset(ones1f[:], 1.0)

        bq_t = cpool.tile([P, EC], F32)
        nc.sync.dma_start(bq_t[:], io["bq"].rearrange("(j p) -> p j", p=P))
        bk_t = cpool.tile([P, EC], F32)
        nc.sync.dma_start(bk_t[:], io["bk"].rearrange("(j p) -> p j", p=P))
        bo_t = cpool.tile([P, EC], F32)
        nc.sync.dma_start(bo_t[:], io["bo"].rearrange("(j p) -> p j", p=P))
        bv_t = cpool.tile([1, E], BF16)
        nc.sync.dma_start(bv_t[:], io["bv"].rearrange("(o e) -> o e", o=1))

        # long-lived activations (all bf16)
        rpool = ctx.enter_context(tc.tile_pool(name="resident", bufs=1))
        qhT = rpool.tile([P, EC, NQ], BF16)          # [dim, q]       8 KB/par
        headcat = rpool.tile([P, EC, NQ], BF16)      # [dim, q]       8 KB/par
        eM = rpool.tile([P, TC, NQ], BF16)           # exp(maskT)    16 KB/par
        m2T = rpool.tile([P, TC, NQ], BF16)          # mask2T        16 KB/par

        dram = ctx.enter_context(tc.tile_pool(name="dram", bufs=1, space="DRAM"))
        dbounce = ctx.enter_context(tc.tile_pool(name="dbounce", bufs=2, space="DRAM"))

        # all input loads have no deps; they stream on the sync queue and are
        # ordered by first use (weights/x first - emitted in the proj block)
        def load_masks(mlp, gs):
            for g in gs:
                mt = mlp.tile([P, 2, NQ], BF16, tag="mt", name="mt")
                nc.scalar.dma_start(
                    mt[:], io["maskT"].rearrange("(c p) q -> p c q", p=P)[:, ts(g, 2), :])
                nc.scalar.activation(eM[:, ts(g, 2), :], mt[:], Exp)

        def load_m2():
            for g in range(TC // 2):
                nc.scalar.dma_start(
                    m2T[:, ts(g, 2), :],
                    io["mask2T"].rearrange("(c p) q -> p c q", p=P)[:, ts(g, 2), :])

        # ---------------- projections (own 512 tokens only) ----------------
        khT_part = dram.tile([E, NQ], BF16)          # this core's khT slice
        vh_part = dram.tile([NQ, E], BF16)           # this core's vh slice
        khT_ag = dram.tile([QB, E, NQ], BF16)
        vh_ag = dram.tile([QB, NQ, E], BF16)

        kvpool = ctx.enter_context(tc.tile_pool(name="kv", bufs=2))

        def load_khp(pair, eng):
            khp = kvpool.tile([P, QB, NQ], BF16, tag="kh", name="khp")
            for g in range(QB):
                eng.dma_start(khp[:, g, :], khT_ag[g, ds(pair * P, P), :])
            return khp

        def load_vhp(pair, eng):
            vhp = kvpool.tile([P, TC, P], BF16, tag="vh", name="vhp", bufs=3)
            for g in range(QB):
                eng.dma_start(
                    vhp[:, ds(g * (TC // QB), TC // QB), :],
                    vh_ag[g].rearrange("(c p) d -> p c d", p=P)[:, :, ds(pair * P, P)])
            return vhp

        with tc.tile_pool(name="wt", bufs=2) as wpool, \
             tc.tile_pool(name="xt", bufs=2) as xpool, \
             tc.tile_pool(name="pout", bufs=2) as opool, \
             tc.tile_pool(name="pps", bufs=4, space="PSUM") as ppsum:

            def load_w(name):
                w_t = wpool.tile([P, EC, E], BF16, tag="w", name="w_t")
                for e in range(EC):
                    nc.sync.dma_start(
                        w_t[:, e, :],
                        io[name].rearrange("(eo p) d -> p eo d", p=P)[:, e, :])
                return w_t

            def load_x(dst, name):
                for e in range(EC):
                    nc.sync.dma_start(
                        dst[:, e, :],
                        io[name].rearrange("(eo p) t -> p eo t", p=P)[:, e, :])

            # k projection -> khT_part, then AllGather early
            wk_t = load_w("wkT")
            xk_t = xpool.tile([P, EC, NQ], BF16, tag="x", name="xk_t")
            load_x(xk_t, "kT")
            kp = opool.tile([P, EC, NQ], BF16, tag="kp", name="kp")
            for j in range(EC):
                ps = ppsum.tile([P, NQ], F32, tag="ps", name="ps")
                for e in range(EC):
                    nc.tensor.matmul(ps[:], wk_t[:, e, ts(j, P)], xk_t[:, e, :],
                                     start=(e == 0), stop=(e == EC - 1))
                nc.scalar.activation(kp[:, j, :], ps[:], Relu, bias=bk_t[:, ds(j, 1)])
                nc.gpsimd.dma_start(
                    khT_part[:].rearrange("(jo p) t -> p jo t", p=P)[:, j, :],
                    kp[:, j, :])
            nc.gpsimd.collective_compute(
                "AllGather", mybir.AluOpType.bypass, replica_groups=GROUPS,
                ins=[khT_part.opt()], outs=[khT_ag.opt()])
            khp0 = load_khp(0, nc.gpsimd)
            load_masks(xpool, range(0, TC // 4))

            # v projection -> vh_part [tokens, dim]; bias rides a rank-1
            # ones-row matmul (it is along the free axis here).
            wv_t = load_w("wvT")
            xv_t = xpool.tile([P, EC, NQ], BF16, tag="x", name="xv_t")
            load_x(xv_t, "vT")
            vp = opool.tile([P, NQ // P, E], BF16, tag="vp", name="vp")
            for tc2 in range(NQ // P):              # 4 token chunks of 128
                for n in range(E // NQ):            # 2 output-dim halves of 512
                    ps = ppsum.tile([P, NQ], F32, tag="ps", name="ps")
                    for e in range(EC):
                        nc.tensor.matmul(ps[:], xv_t[:, e, ts(tc2, P)],
                                         wv_t[:, e, ts(n, NQ)],
                                         start=(e == 0), stop=False)
                    nc.tensor.matmul(ps[:], ones1b[:], bv_t[:, ts(n, NQ)],
                                     start=False, stop=True)
                    nc.scalar.activation(vp[:, tc2, ts(n, NQ)], ps[:], Relu)
                    nc.gpsimd.dma_start(
                        vh_part[:].rearrange("(c p) d -> p c d", p=P)[:, tc2, ts(n, NQ)],
                        vp[:, tc2, ts(n, NQ)])
            nc.gpsimd.collective_compute(
                "AllGather", mybir.AluOpType.bypass, replica_groups=GROUPS,
                ins=[vh_part.opt()], outs=[vh_ag.opt()])
            vhp0 = load_vhp(0, nc.gpsimd)
            load_masks(xpool, range(TC // 4, TC // 2))

            # q projection -> qhT resident
            wq_t = load_w("wqT")
            xq_t = xpool.tile([P, EC, NQ], BF16, tag="x", name="xq_t")
            load_x(xq_t, "qT")
            for j in range(EC):
                ps = ppsum.tile([P, NQ], F32, tag="ps", name="ps")
                for e in range(EC):
                    nc.tensor.matmul(ps[:], wq_t[:, e, ts(j, P)], xq_t[:, e, :],
                                     start=(e == 0), stop=(e == EC - 1))
                nc.scalar.activation(qhT[:, j, :], ps[:], Relu, bias=bq_t[:, ds(j, 1)])
            load_m2()
            # keep the PE (and its HAM clock) warm while the k-AllGather
            # finishes - it has no other work until khp(0) lands
            heat = ppsum.tile([1, NQ], F32, tag="heat", name="heat")
            for _ in range(190):
                nc.tensor.matmul(heat[:], ones128[:], kp[:, 0, :],
                                 start=True, stop=True, skip_group_check=True)

        # out-proj weights load early on the sync queue (used only at the tail)
        wopool = ctx.enter_context(tc.tile_pool(name="wo", bufs=1))
        wo_t = wopool.tile([P, EC, E], BF16)
        for e in range(EC):
            nc.sync.dma_start(
                wo_t[:, e, :],
                io["woT"].rearrange("(eo p) d -> p eo d", p=P)[:, e, :])

        # ---------------- attention ----------------
        with tc.tile_pool(name="p", bufs=2) as ppool, \
             tc.tile_pool(name="work", bufs=3) as wk, \
             tc.tile_pool(name="invd", bufs=2) as ivpool, \
             tc.tile_pool(name="sps", bufs=2, space="PSUM") as spsum, \
             tc.tile_pool(name="dps", bufs=1, space="PSUM") as dpsum, \
             tc.tile_pool(name="ops", bufs=1, space="PSUM") as opsum:

            spart = [ds(0, D), ds(D, D)]

            def emit_av_prologue(prev):
                """broadcast 1/d (tiny rank-1 matmuls; inputs long ready)."""
                ib = []
                for hh in range(2):
                    b_ps = dpsum.tile([P, NQ], F32, tag=f"d{hh}", name=f"b_ps{hh}")
                    nc.tensor.matmul(b_ps[:], ones1f[:], prev[2][hh][:],
                                     start=True, stop=True)
                    ib_t = ivpool.tile([P, NQ], BF16, tag=f"ib{hh}", name=f"ib{hh}")
                    nc.scalar.copy(ib_t[:], b_ps[:])
                    ib.append(ib_t)
                # both heads share one PSUM bank, split on the partition axis
                o_ps = opsum.tile([P, NQ], F32, tag="o", name="o_ps")
                return ib, o_ps

            def emit_av_block(prev, ib, o_ps, g):
                """normalize + mask2 + relu + attn@v for chunk-pair g of the
                PREVIOUS head pair (interleaved into the current pair)."""
                _, p_prev, _, vhp_prev = prev
                for hh in range(2):
                    ibb = ib[hh][:, None, :].broadcast_to([P, 2, NQ])
                    half = p_prev[2 * hh + g // 4]
                    p2 = wk.tile([P, 2, NQ], BF16, tag="p2", name="p2")
                    nc.vector.tensor_mul(p2[:], half[:, ts(g % 4, 2), :], ibb)
                    w_t = wk.tile([P, 2, NQ], BF16, tag="w", name="w_t")
                    nc.vector.tensor_add(w_t[:], p2[:], m2T[:, ts(g, 2), :])
                    nc.vector.tensor_scalar_max(w_t[:], w_t[:], 0.0)
                    for cc in range(2):
                        c = 2 * g + cc
                        nc.tensor.matmul(o_ps[ds(hh * D, D), :],
                                         vhp_prev[:, c, ds(hh * D, D)],
                                         w_t[:, cc, :],
                                         start=(c == 0), stop=(c == TC - 1),
                                         skip_group_check=True)

            def emit_av_epilogue(prev, o_ps):
                nc.scalar.copy(headcat[:, prev[0], :], o_ps[:])

            def emit_d_chunks(d_ps, p_t, g):
                """ones-matmul accumulation of chunks 2g, 2g+1 for both heads."""
                for hh in range(2):
                    for cc in range(2):
                        c = 2 * g + cc
                        half = p_t[2 * hh + g // 4]
                        nc.tensor.matmul(d_ps[hh][:], ones128[:],
                                         half[:, (g % 4) * 2 + cc, :],
                                         start=(c == 0), stop=(c == TC - 1))

            def emit_bounce(d_ps):
                """PSUM d (both heads) -> reciprocal -> [1, NQ] 1/d each, via
                one merged DRAM reshape round-trip."""
                d_sb = ivpool.tile([1, 2 * NQ], F32, tag="dsb", name="dsb")
                for hh in range(2):
                    nc.scalar.copy(d_sb[:, ds(hh * NQ, NQ)], d_ps[hh][:])
                d_dram = dbounce.tile([2 * NQ], F32, tag="dd", name="dd")
                nc.gpsimd.dma_start(d_dram[:].rearrange("(o q) -> o q", o=1), d_sb[:])
                d_r = ivpool.tile([P, 2 * NQ // P], F32, tag="dr", name="dr")
                nc.gpsimd.dma_start(d_r[:], d_dram[:].rearrange("(p f) -> p f", p=P))
                iv_r = ivpool.tile([P, 2 * NQ // P], F32, tag="ivr", name="ivr")
                nc.vector.reciprocal(iv_r[:], d_r[:])
                iv_dram = dbounce.tile([2 * NQ], F32, tag="ivd", name="ivd")
                nc.gpsimd.dma_start(iv_dram[:].rearrange("(p f) -> p f", p=P), iv_r[:])
                iv = []
                for hh in range(2):
                    iv_f = ivpool.tile([1, NQ], F32, tag=f"ivf{hh}", name=f"ivf{hh}")
                    nc.gpsimd.dma_start(
                        iv_f[:],
                        iv_dram[ds(hh * NQ, NQ)].rearrange("(o q) -> o q", o=1))
                    iv.append(iv_f)
                return iv

            prev = None
            nextkv = (khp0, vhp0)
            for pair in range(H // 2):               # two heads per 128-row block
                khp, vhp = nextkv
                if pair + 1 < H // 2:
                    nextkv = (load_khp(pair + 1, nc.sync),
                              load_vhp(pair + 1, nc.sync))
                # p split into half-pair tiles so pair N+1's scores don't wait
                # on the full consumption of pair N-1's p
                p_t = [ppool.tile([P, TC // 2, NQ], BF16, tag=f"p{hh}{ab}",
                                  name=f"p{hh}{ab}")
                       for hh in range(2) for ab in range(2)]
                d_ps = [dpsum.tile([1, NQ], F32, tag=f"d{hh}", name=f"d_ps{hh}")
                        for hh in range(2)]
                # chunk-interleaved emission: the PE queue alternates between
                # scores (gated by exp recycling s_ps), the d ones-matmuls
                # (gated by exp one chunk back), and the previous pair's
                # attn@v (inputs all ready) - so it never stalls.
                for g in range(TC // 2):
                    for hh in range(2):
                        s_ps = spsum.tile([P, 2, NQ], F32, tag="s", name="s_ps")
                        for cc in range(2):
                            c = 2 * g + cc
                            nc.tensor.matmul(
                                s_ps[:, cc, :],
                                khp[spart[hh], c // QB, ds((c % QB) * P, P)],
                                qhT[spart[hh], pair, :], start=True, stop=True)
                        half = p_t[2 * hh + g // 4]
                        psl = half[:, ts(g % 4, 2), :]
                        nc.scalar.activation(psl, s_ps[:], Exp, scale=SCALE)
                        nc.vector.tensor_mul(psl, psl, eM[:, ts(g, 2), :])
                    if g == 1 and prev is not None:
                        ib, o_ps = emit_av_prologue(prev)
                    if g >= 1:
                        emit_d_chunks(d_ps, p_t, g - 1)
                    if g >= 2 and prev is not None:
                        emit_av_block(prev, ib, o_ps, g - 2)
                emit_d_chunks(d_ps, p_t, TC // 2 - 1)
                iv = emit_bounce(d_ps)
                if prev is not None:
                    emit_av_block(prev, ib, o_ps, TC // 2 - 2)
                    emit_av_block(prev, ib, o_ps, TC // 2 - 1)
                    emit_av_epilogue(prev, o_ps)
                prev = (pair, p_t, iv, vhp)
            ib, o_ps = emit_av_prologue(prev)
            for g in range(TC // 2):
                emit_av_block(prev, ib, o_ps, g)
            emit_av_epilogue(prev, o_ps)

        # ---------------- output projection ----------------
        with tc.tile_pool(name="ops2", bufs=4, space="PSUM") as opsum2, \
             tc.tile_pool(name="oout", bufs=4) as oopool:
            for j in range(EC):
                ps = opsum2.tile([P, NQ], F32, tag="ps", name="ps")
                for e in range(EC):
                    nc.tensor.matmul(ps[:], wo_t[:, e, ts(j, P)], headcat[:, e, :],
                                     start=(e == 0), stop=(e == EC - 1))
                o_t = oopool.tile([P, NQ], F32, tag="o", name="o_t")
                nc.scalar.activation(o_t[:], ps[:], Relu, bias=bo_t[:, ds(j, 1)])
                nc.sync.dma_start(
                    io["outT"].rearrange("(jo p) q -> p jo q", p=P)[:, j, :], o_t[:])


_PROGRAM = None


def _build_program():
    global _PROGRAM
    if _PROGRAM is not None:
        return _PROGRAM
    nc = bacc.Bacc("TRN2", target_bir_lowering=False, debug=False,
                   num_devices=NCORES)
    io = {}
    def inp(name, shape, dt=BF16):
        io[name] = nc.dram_tensor(name, shape, dt, kind="ExternalInput").ap()
    inp("qT", [E, NQ])
    inp("kT", [E, NQ])
    inp("vT", [E, NQ])
    inp("maskT", [S, NQ])
    inp("mask2T", [S, NQ])
    for w in ("wqT", "wkT", "wvT", "woT"):
        inp(w, [E, E])
    for b in ("bq", "bk", "bo"):
        inp(b, [E], F32)
    inp("bv", [E], BF16)
    io["outT"] = nc.dram_tensor("outT", [E, NQ], F32, kind="ExternalOutput").ap()

    with tile.TileContext(nc) as tc:
        _emit(tc, io)
    nc.compile()
    _PROGRAM = (nc, io)
    return _PROGRAM


def kernel(q, k, v, mask, mask2, Wq, bq, Wk, bk, Wv, bv, Wo, bo, _trace=False):
    nc, _ = _build_program()

    def bf(x):
        return np.ascontiguousarray(x, dtype=NPBF)

    wqT = bf(Wq.T)
    wkT = bf(Wk.T)
    wvT = bf(Wv.T)
    woT = bf(Wo.T)

    in_maps = []
    for c in range(NCORES):
        b, qb = divmod(c, QB)
        rows = slice(qb * NQ, (qb + 1) * NQ)
        in_maps.append({
            "qT": bf(q[b, rows, :].T),
            "kT": bf(k[b, rows, :].T),
            "vT": bf(v[b, rows, :].T),
            "maskT": bf(mask[b, rows, :].T),
            "mask2T": bf(mask2[b, rows, :].T),
            "wqT": wqT, "wkT": wkT, "wvT": wvT, "woT": woT,
            "bq": np.ascontiguousarray(bq, dtype=np.float32),
            "bk": np.ascontiguousarray(bk, dtype=np.float32),
            "bo": np.ascontiguousarray(bo, dtype=np.float32),
            "bv": bf(bv),
        })

    res = run_bass_kernel_spmd(nc, in_maps, core_ids=list(range(NCORES)),
                               trace=_trace)

    out = np.empty((B, S, E), dtype=np.float32)
    for c in range(NCORES):
        b, qb = divmod(c, QB)
        out[b, qb * NQ:(qb + 1) * NQ, :] = res.results[c]["outT"].T
    if _trace:
        kernel.last_results = res
    return out
